# revision 10
# baseline (speedup 1.0000x reference)
"""MoE actor-critic forward kernel for 8 Trainium2 NeuronCores.

Strategy: data-parallel over the batch axis. Each of the 8 cores gets
B/8 = 2048 tokens plus a full replica of the gating + expert weights.

Per-core math (all activations feature-major [feat_part, tok] in SBUF):
  - gating MLP in fp32 (exact routing): 512 -> 256 -> 128 -> 8 logits
    token-major, then top-2 + renormalized combine weights cw[tok, 8]
    computed with exp/compare/reduce ops (no explicit softmax division
    needed: cw = z * (eq1+eq2) / (1 + m2) with z = exp(l - lmax)).
  - experts in bf16 (fp32 PSUM accumulate): all 8 experts run densely,
    512 -> 1024 -> 512 -> 256 -> 32 with ELU between layers.
    ELU(x) = max(x + b, min(exp(x + b) - 1, 0)) -> 1 ACT + 2 DVE ops.
  - combine: acc[tok, 32] += cw[:, e] * (h3_e @ W4_e + b4_e), where the
    bias enters the matmul via a K=1 ones-row matmul.
"""

import numpy as np
import ml_dtypes

import concourse.bass as bass
import concourse.mybir as mybir
import concourse.tile as tile
from concourse import bacc
from concourse.bass_utils import run_bass_kernel_spmd

BF16 = mybir.dt.bfloat16
F32 = mybir.dt.float32
NP_BF16 = ml_dtypes.bfloat16

B = 16384
D = 512          # obs dim
A = 32           # actions
E = 8            # experts
NCORES = 8
T = B // NCORES  # tokens per core (2048)
NT = T // 512    # 512-token tiles (4)
TT = T // 128    # 128-token tiles (16)

EH1, EH2, EH3 = 1024, 512, 256
GH1, GH2 = 256, 128

LAST_RESULTS = None  # test harness reads exec_time_ns from here


def _build_bass():
    nc = bacc.Bacc("TRN2", target_bir_lowering=False, debug=False,
                   enable_asserts=False, num_devices=NCORES)

    # ---- DRAM I/O ----
    obs_f = nc.dram_tensor("obs_f", [D, T], F32, kind="ExternalInput")
    obs_b = nc.dram_tensor("obs_b", [D, T], BF16, kind="ExternalInput")
    gw1 = nc.dram_tensor("gw1", [D, GH1], F32, kind="ExternalInput")
    gw2 = nc.dram_tensor("gw2", [GH1, GH2], F32, kind="ExternalInput")
    gw3 = nc.dram_tensor("gw3", [GH2, E], F32, kind="ExternalInput")
    gb1 = nc.dram_tensor("gb1", [128, GH1 // 128], F32, kind="ExternalInput")
    gb2 = nc.dram_tensor("gb2", [128, GH2 // 128], F32, kind="ExternalInput")
    gb3 = nc.dram_tensor("gb3", [1, E], F32, kind="ExternalInput")
    ew1 = nc.dram_tensor("ew1", [E, D, EH1], BF16, kind="ExternalInput")
    ew2 = nc.dram_tensor("ew2", [E, EH1, EH2], BF16, kind="ExternalInput")
    ew3 = nc.dram_tensor("ew3", [E, EH2, EH3], BF16, kind="ExternalInput")
    ew4 = nc.dram_tensor("ew4", [E, EH3, A], BF16, kind="ExternalInput")
    eb1 = nc.dram_tensor("eb1", [E, 128, EH1 // 128], F32, kind="ExternalInput")
    eb2 = nc.dram_tensor("eb2", [E, 128, EH2 // 128], F32, kind="ExternalInput")
    eb3 = nc.dram_tensor("eb3", [E, 128, EH3 // 128], F32, kind="ExternalInput")
    eb4 = nc.dram_tensor("eb4", [1, E, A], BF16, kind="ExternalInput")
    out = nc.dram_tensor("out", [T, A], F32, kind="ExternalOutput")

    with tile.TileContext(nc) as tc:
        _emit(nc, tc, obs_f, obs_b, gw1, gw2, gw3, gb1, gb2, gb3,
              ew1, ew2, ew3, ew4, eb1, eb2, eb3, eb4, out)
    nc.compile()
    return nc


def _elu(nc, pool, psum, bias_col, h_out):
    """h_out = ELU(psum + bias_col) = max(x+b, min(exp(x+b)-1, 0))."""
    p, n = psum.shape[0], psum.free_size()
    t = pool.tile([128, 512], BF16, tag="elu_t")
    u = pool.tile([128, 512], BF16, tag="elu_u")
    nc.scalar.activation(t[:p, :n], psum, mybir.ActivationFunctionType.Exp,
                         bias=bias_col)
    nc.vector.tensor_scalar(u[:p, :n], t[:p, :n], -1.0, 0.0,
                            mybir.AluOpType.add, mybir.AluOpType.min)
    nc.vector.scalar_tensor_tensor(h_out, psum, bias_col, u[:p, :n],
                                   mybir.AluOpType.add, mybir.AluOpType.max)


def _emit(nc, tc, obs_f, obs_b, gw1, gw2, gw3, gb1, gb2, gb3,
          ew1, ew2, ew3, ew4, eb1, eb2, eb3, eb4, out):
    AF = mybir.ActivationFunctionType
    OP = mybir.AluOpType
    X = mybir.AxisListType.X

    # ---------------- persistent pools ----------------
    from contextlib import ExitStack
    ctx = ExitStack()
    consts = ctx.enter_context(tc.tile_pool(name="consts", bufs=1))
    acts = ctx.enter_context(tc.tile_pool(name="acts", bufs=1))
    wpool = ctx.enter_context(tc.tile_pool(name="wpool", bufs=2))
    tmp = ctx.enter_context(tc.tile_pool(name="tmp", bufs=4))
    psum_mm = ctx.enter_context(tc.tile_pool(name="psum_mm", bufs=4, space="PSUM"))

    # constants
    ones_b = consts.tile([1, 128], BF16)
    nc.vector.memset(ones_b, 1.0)
    ones_f = consts.tile([1, 128], F32)
    nc.vector.memset(ones_f, 1.0)
    b4_sb = consts.tile([1, E, A], BF16)
    nc.sync.dma_start(out=b4_sb, in_=eb4[:, :, :])
    gb3_sb = consts.tile([1, E], F32)
    nc.sync.dma_start(out=gb3_sb, in_=gb3[:, :])

    # persistent activations (feature-major, bf16)
    obs_sb = acts.tile([128, D // 128, T], BF16)
    nc.sync.dma_start(out=obs_sb, in_=obs_b.rearrange("(k p) t -> p k t", p=128))
    h1 = acts.tile([128, EH1 // 128, T], BF16)
    h2 = acts.tile([128, EH2 // 128, T], BF16)
    h3 = acts.tile([128, EH3 // 128, T], BF16)
    cw = acts.tile([128, TT, E], F32)          # combine weights, token-major
    acc = acts.tile([128, TT, A], F32)         # final output accumulator

    # ---------------- gating (fp32) ----------------
    with tc.tile_pool(name="gating", bufs=1) as gp, \
         tc.tile_pool(name="gstream", bufs=2) as gs, \
         tc.tile_pool(name="gtmp", bufs=4) as gt, \
         tc.tile_pool(name="psum_g", bufs=2, space="PSUM") as pg:

        gw1_sb = gp.tile([128, D // 128, GH1], F32)
        nc.sync.dma_start(out=gw1_sb, in_=gw1.rearrange("(k p) o -> p k o", p=128))
        gw2_sb = gp.tile([128, GH1 // 128, GH2], F32)
        nc.sync.dma_start(out=gw2_sb, in_=gw2.rearrange("(k p) o -> p k o", p=128))
        gw3_sb = gp.tile([128, E], F32)
        nc.sync.dma_start(out=gw3_sb, in_=gw3[:, :])
        gb1_sb = gp.tile([128, GH1 // 128], F32)
        nc.sync.dma_start(out=gb1_sb, in_=gb1[:, :])
        gb2_sb = gp.tile([128, GH2 // 128], F32)
        nc.sync.dma_start(out=gb2_sb, in_=gb2[:, :])
        g1 = gp.tile([128, GH1 // 128, T], F32)
        g2 = gp.tile([128, GH2 // 128, T], F32)

        # L1: 512 -> 256
        for n in range(NT):
            ob = gs.tile([128, D // 128, 512], F32, tag="gobs")
            nc.sync.dma_start(
                out=ob,
                in_=obs_f.rearrange("(k p) t -> p k t", p=128)[:, :, n * 512:(n + 1) * 512])
            for m in range(GH1 // 128):
                ps = pg.tile([128, 512], F32, tag="gps")
                for k in range(D // 128):
                    nc.tensor.matmul(ps, gw1_sb[:, k, m * 128:(m + 1) * 128],
                                     ob[:, k, :], start=(k == 0), stop=(k == D // 128 - 1))
                _elu_g(nc, gt, ps, gb1_sb[:, m:m + 1], g1[:, m, n * 512:(n + 1) * 512])
        # L2: 256 -> 128
        for n in range(NT):
            ps = pg.tile([128, 512], F32, tag="gps")
            for k in range(GH1 // 128):
                nc.tensor.matmul(ps, gw2_sb[:, k, :], g1[:, k, n * 512:(n + 1) * 512],
                                 start=(k == 0), stop=(k == GH1 // 128 - 1))
            _elu_g(nc, gt, ps, gb2_sb[:, 0:1], g2[:, 0, n * 512:(n + 1) * 512])
        # logits + top-2 combine weights, token-major per 128-token tile
        for t in range(TT):
            pl = pg.tile([128, E], F32, tag="gpl")
            nc.tensor.matmul(pl, g2[:, 0, t * 128:(t + 1) * 128], gw3_sb,
                             start=True, stop=False)
            nc.tensor.matmul(pl, ones_f, gb3_sb, start=False, stop=True)

            mx = gt.tile([128, 1], F32, tag="mx")
            nmx = gt.tile([128, 1], F32, tag="nmx")
            z = gt.tile([128, E], F32, tag="z")
            eq1 = gt.tile([128, E], F32, tag="eq1")
            z2 = gt.tile([128, E], F32, tag="z2")
            m2 = gt.tile([128, 1], F32, tag="m2")
            eq2 = gt.tile([128, E], F32, tag="eq2")
            msk = gt.tile([128, E], F32, tag="msk")
            num = gt.tile([128, E], F32, tag="num")
            den = gt.tile([128, 1], F32, tag="den")
            rec = gt.tile([128, 1], F32, tag="rec")

            nc.vector.reduce_max(out=mx, in_=pl, axis=X)
            nc.vector.tensor_scalar_mul(nmx, mx, -1.0)
            nc.vector.tensor_scalar(eq1, pl, mx, None, OP.is_ge)
            nc.scalar.activation(z, pl, AF.Exp, bias=nmx)
            nc.vector.tensor_sub(z2, z, eq1)
            nc.vector.reduce_max(out=m2, in_=z2, axis=X)
            nc.vector.tensor_scalar(eq2, z2, m2, None, OP.is_ge)
            nc.vector.tensor_add(msk, eq1, eq2)
            nc.vector.tensor_mul(num, z, msk)
            nc.vector.tensor_scalar_add(den, m2, 1.0)
            nc.vector.reciprocal(rec, den)
            nc.vector.tensor_scalar_mul(cw[:, t, :], num, rec)

    # ---------------- experts (bf16) ----------------
    for e in range(E):
        w1_sb = wpool.tile([128, D // 128, EH1], BF16, tag="w1")
        nc.sync.dma_start(out=w1_sb, in_=ew1[e].rearrange("(k p) o -> p k o", p=128))
        w2_sb = wpool.tile([128, EH1 // 128, EH2], BF16, tag="w2")
        nc.sync.dma_start(out=w2_sb, in_=ew2[e].rearrange("(k p) o -> p k o", p=128))
        w3_sb = wpool.tile([128, EH2 // 128, EH3], BF16, tag="w3")
        nc.sync.dma_start(out=w3_sb, in_=ew3[e].rearrange("(k p) o -> p k o", p=128))
        w4_sb = wpool.tile([128, EH3 // 128, A], BF16, tag="w4")
        nc.sync.dma_start(out=w4_sb, in_=ew4[e].rearrange("(k p) o -> p k o", p=128))
        b1_sb = wpool.tile([128, EH1 // 128], F32, tag="b1")
        nc.sync.dma_start(out=b1_sb, in_=eb1[e])
        b2_sb = wpool.tile([128, EH2 // 128], F32, tag="b2")
        nc.sync.dma_start(out=b2_sb, in_=eb2[e])
        b3_sb = wpool.tile([128, EH3 // 128], F32, tag="b3")
        nc.sync.dma_start(out=b3_sb, in_=eb3[e])

        # L1: 512 -> 1024
        for n in range(NT):
            for m in range(EH1 // 128):
                ps = psum_mm.tile([128, 512], F32, tag="mm")
                for k in range(D // 128):
                    nc.tensor.matmul(ps, w1_sb[:, k, m * 128:(m + 1) * 128],
                                     obs_sb[:, k, n * 512:(n + 1) * 512],
                                     start=(k == 0), stop=(k == D // 128 - 1))
                _elu(nc, tmp, ps, b1_sb[:, m:m + 1], h1[:, m, n * 512:(n + 1) * 512])
        # L2: 1024 -> 512
        for n in range(NT):
            for m in range(EH2 // 128):
                ps = psum_mm.tile([128, 512], F32, tag="mm")
                for k in range(EH1 // 128):
                    nc.tensor.matmul(ps, w2_sb[:, k, m * 128:(m + 1) * 128],
                                     h1[:, k, n * 512:(n + 1) * 512],
                                     start=(k == 0), stop=(k == EH1 // 128 - 1))
                _elu(nc, tmp, ps, b2_sb[:, m:m + 1], h2[:, m, n * 512:(n + 1) * 512])
        # L3: 512 -> 256
        for n in range(NT):
            for m in range(EH3 // 128):
                ps = psum_mm.tile([128, 512], F32, tag="mm")
                for k in range(EH2 // 128):
                    nc.tensor.matmul(ps, w3_sb[:, k, m * 128:(m + 1) * 128],
                                     h2[:, k, n * 512:(n + 1) * 512],
                                     start=(k == 0), stop=(k == EH2 // 128 - 1))
                _elu(nc, tmp, ps, b3_sb[:, m:m + 1], h3[:, m, n * 512:(n + 1) * 512])
        # L4 + weighted combine: 256 -> 32, token-major
        for t in range(TT):
            ps = psum_mm.tile([128, 512], F32, tag="mm")
            p4 = ps[:, :A]
            for k in range(EH3 // 128):
                nc.tensor.matmul(p4, h3[:, k, t * 128:(t + 1) * 128], w4_sb[:, k, :],
                                 start=(k == 0), stop=False)
            nc.tensor.matmul(p4, ones_b, b4_sb[:, e, :], start=False, stop=True)
            if e == 0:
                nc.vector.tensor_scalar(acc[:, t, :], p4, cw[:, t, e:e + 1], None,
                                        OP.mult)
            else:
                nc.vector.scalar_tensor_tensor(acc[:, t, :], p4, cw[:, t, e:e + 1],
                                               acc[:, t, :], OP.mult, OP.add)

    # ---------------- store ----------------
    nc.sync.dma_start(out=out.rearrange("(t p) a -> p t a", p=128), in_=acc)

    ctx.close()


def _elu_g(nc, pool, psum, bias_col, h_out):
    """fp32 ELU for the gating net."""
    t = pool.tile([128, 512], F32, tag="gelu_t")
    u = pool.tile([128, 512], F32, tag="gelu_u")
    n = psum.free_size()
    nc.scalar.activation(t[:, :n], psum, mybir.ActivationFunctionType.Exp,
                         bias=bias_col)
    nc.vector.tensor_scalar(u[:, :n], t[:, :n], -1.0, 0.0,
                            mybir.AluOpType.add, mybir.AluOpType.min)
    nc.vector.scalar_tensor_tensor(h_out, psum, bias_col, u[:, :n],
                                   mybir.AluOpType.add, mybir.AluOpType.max)


_CACHED_NC = None


def kernel(**inputs) -> np.ndarray:
    global LAST_RESULTS, _CACHED_NC
    obs = np.ascontiguousarray(inputs["observations"], dtype=np.float32)

    def pp_bias(b):  # [chunks*128] -> [128, chunks] per-partition layout
        c = b.shape[-1] // 128
        return np.ascontiguousarray(
            b.reshape(b.shape[:-1] + (c, 128)).swapaxes(-1, -2), dtype=np.float32)

    gw1 = np.asarray(inputs["gw1"], np.float32)
    gw2 = np.asarray(inputs["gw2"], np.float32)
    gw3 = np.asarray(inputs["gw3"], np.float32)
    gb1 = pp_bias(np.asarray(inputs["gb1"], np.float32))
    gb2 = pp_bias(np.asarray(inputs["gb2"], np.float32))
    gb3 = np.asarray(inputs["gb3"], np.float32).reshape(1, E)
    ew1 = np.ascontiguousarray(inputs["ew1"], dtype=np.float32).astype(NP_BF16)
    ew2 = np.ascontiguousarray(inputs["ew2"], dtype=np.float32).astype(NP_BF16)
    ew3 = np.ascontiguousarray(inputs["ew3"], dtype=np.float32).astype(NP_BF16)
    ew4 = np.ascontiguousarray(inputs["ew4"], dtype=np.float32).astype(NP_BF16)
    eb1 = pp_bias(np.asarray(inputs["eb1"], np.float32))
    eb2 = pp_bias(np.asarray(inputs["eb2"], np.float32))
    eb3 = pp_bias(np.asarray(inputs["eb3"], np.float32))
    eb4 = np.asarray(inputs["eb4"], np.float32).reshape(1, E, A).astype(NP_BF16)

    shared = {
        "gw1": gw1, "gw2": gw2, "gw3": gw3,
        "gb1": gb1, "gb2": gb2, "gb3": gb3,
        "ew1": ew1, "ew2": ew2, "ew3": ew3, "ew4": ew4,
        "eb1": eb1, "eb2": eb2, "eb3": eb3, "eb4": eb4,
    }
    in_maps = []
    for c in range(NCORES):
        sl = obs[c * T:(c + 1) * T]                    # [T, D]
        obs_t = np.ascontiguousarray(sl.T)             # [D, T] fp32
        m = dict(shared)
        m["obs_f"] = obs_t
        m["obs_b"] = obs_t.astype(NP_BF16)
        in_maps.append(m)

    if _CACHED_NC is None:
        _CACHED_NC = _build_bass()
    nc = _CACHED_NC

    LAST_RESULTS = run_bass_kernel_spmd(nc, in_maps, core_ids=list(range(NCORES)))
    outs = [LAST_RESULTS.results[c]["out"] for c in range(NCORES)]
    return np.concatenate(outs, axis=0).astype(np.float32)


# revision 18
# speedup vs baseline: 1.0566x; 1.0566x over previous
"""MoE actor-critic forward kernel for 8 Trainium2 NeuronCores.

Strategy: data-parallel over the batch axis. Each of the 8 cores gets
B/8 = 2048 tokens plus a full replica of the gating + expert weights.

Per-core math (all activations feature-major [feat_part, tok] in SBUF):
  - gating MLP in fp32 (exact routing): 512 -> 256 -> 128 -> 8 logits
    token-major, then top-2 + renormalized combine weights cw[tok, 8]
    computed with exp/compare/reduce ops (no explicit softmax division
    needed: cw = z * (eq1+eq2) / (1 + m2) with z = exp(l - lmax)).
  - experts in bf16 (fp32 PSUM accumulate): all 8 experts run densely,
    512 -> 1024 -> 512 -> 256 -> 32 with ELU between layers.
    ELU(x) = max(x + b, min(exp(x + b) - 1, 0)) -> 1 ACT + 2 DVE ops.
  - combine: acc[tok, 32] += cw[:, e] * (h3_e @ W4_e + b4_e), where the
    bias enters the matmul via a K=1 ones-row matmul.
"""

import numpy as np
import ml_dtypes

import concourse.bass as bass
import concourse.mybir as mybir
import concourse.tile as tile
from concourse import bacc
from concourse.bass_utils import run_bass_kernel_spmd

BF16 = mybir.dt.bfloat16
F32 = mybir.dt.float32
NP_BF16 = ml_dtypes.bfloat16

B = 16384
D = 512          # obs dim
A = 32           # actions
E = 8            # experts
NCORES = 8
T = B // NCORES  # tokens per core (2048)
NT = T // 512    # 512-token tiles (4)
TT = T // 128    # 128-token tiles (16)

EH1, EH2, EH3 = 1024, 512, 256
GH1, GH2 = 256, 128

LAST_RESULTS = None  # test harness reads exec_time_ns from here


def _build_bass():
    nc = bacc.Bacc("TRN2", target_bir_lowering=False, debug=False,
                   enable_asserts=False, num_devices=NCORES)

    # ---- DRAM I/O ----
    obs_f = nc.dram_tensor("obs_f", [D, T], F32, kind="ExternalInput")
    obs_b = nc.dram_tensor("obs_b", [D, T], BF16, kind="ExternalInput")
    gw1 = nc.dram_tensor("gw1", [D, GH1], F32, kind="ExternalInput")
    gw2 = nc.dram_tensor("gw2", [GH1, GH2], F32, kind="ExternalInput")
    gw3 = nc.dram_tensor("gw3", [GH2, E], F32, kind="ExternalInput")
    gb1 = nc.dram_tensor("gb1", [128, GH1 // 128], F32, kind="ExternalInput")
    gb2 = nc.dram_tensor("gb2", [128, GH2 // 128], F32, kind="ExternalInput")
    gb3 = nc.dram_tensor("gb3", [1, E], F32, kind="ExternalInput")
    ew1 = nc.dram_tensor("ew1", [E, D, EH1], BF16, kind="ExternalInput")
    ew2 = nc.dram_tensor("ew2", [E, EH1, EH2], BF16, kind="ExternalInput")
    ew3 = nc.dram_tensor("ew3", [E, EH2, EH3], BF16, kind="ExternalInput")
    ew4 = nc.dram_tensor("ew4", [E, EH3, A], BF16, kind="ExternalInput")
    eb1 = nc.dram_tensor("eb1", [E, 128, EH1 // 128], F32, kind="ExternalInput")
    eb2 = nc.dram_tensor("eb2", [E, 128, EH2 // 128], F32, kind="ExternalInput")
    eb3 = nc.dram_tensor("eb3", [E, 128, EH3 // 128], F32, kind="ExternalInput")
    eb4 = nc.dram_tensor("eb4", [1, E, A], F32, kind="ExternalInput")
    out = nc.dram_tensor("out", [T, A], F32, kind="ExternalOutput")

    with tile.TileContext(nc) as tc:
        _emit(nc, tc, obs_f, obs_b, gw1, gw2, gw3, gb1, gb2, gb3,
              ew1, ew2, ew3, ew4, eb1, eb2, eb3, eb4, out)
    nc.compile()
    return nc


_ELU_FLIP = [0]


def _elu(nc, pool, psum, bias_col, h_out):
    """h_out = ELU(psum + bias_col) = max(x+b, min(exp(x+b)-1, 0)).

    The min(exp-1, 0) op alternates between GPSIMD and DVE so neither
    engine becomes the bottleneck.
    """
    p, n = psum.shape[0], psum.free_size()
    t = pool.tile([128, 512], BF16, tag="elu_t")
    u = pool.tile([128, 512], BF16, tag="elu_u")
    nc.scalar.activation(t[:p, :n], psum, mybir.ActivationFunctionType.Exp,
                         bias=bias_col)
    eng = nc.gpsimd if _ELU_FLIP[0] % 2 == 0 else nc.vector
    _ELU_FLIP[0] += 1
    eng.tensor_scalar(u[:p, :n], t[:p, :n], -1.0, 0.0,
                      mybir.AluOpType.add, mybir.AluOpType.min)
    nc.vector.scalar_tensor_tensor(h_out, psum, bias_col, u[:p, :n],
                                   mybir.AluOpType.add, mybir.AluOpType.max)


def _emit(nc, tc, obs_f, obs_b, gw1, gw2, gw3, gb1, gb2, gb3,
          ew1, ew2, ew3, ew4, eb1, eb2, eb3, eb4, out):
    AF = mybir.ActivationFunctionType
    OP = mybir.AluOpType
    X = mybir.AxisListType.X

    # ---------------- persistent pools ----------------
    from contextlib import ExitStack
    ctx = ExitStack()
    consts = ctx.enter_context(tc.tile_pool(name="consts", bufs=1))
    acts = ctx.enter_context(tc.tile_pool(name="acts", bufs=1))
    wpool = ctx.enter_context(tc.tile_pool(name="wpool", bufs=2))
    tmp = ctx.enter_context(tc.tile_pool(name="tmp", bufs=4))
    psum_mm = ctx.enter_context(tc.tile_pool(name="psum_mm", bufs=6, space="PSUM"))

    # constants
    ones_b = consts.tile([1, 128], BF16)
    nc.vector.memset(ones_b, 1.0)
    ones_f = consts.tile([1, 128], F32)
    nc.vector.memset(ones_f, 1.0)
    b4_sb = consts.tile([1, E, A], F32)
    nc.sync.dma_start(out=b4_sb, in_=eb4[:, :, :])
    gb3_sb = consts.tile([1, E], F32)
    nc.sync.dma_start(out=gb3_sb, in_=gb3[:, :])

    # persistent activations (feature-major, bf16)
    obs_sb = acts.tile([128, D // 128, T], BF16)
    nc.sync.dma_start(out=obs_sb, in_=obs_b.rearrange("(k p) t -> p k t", p=128))
    h1 = acts.tile([128, EH1 // 128, T], BF16)
    h2 = acts.tile([128, EH2 // 128, T], BF16)
    h3 = acts.tile([128, EH3 // 128, T], BF16)
    cw = acts.tile([128, TT, E], F32)          # combine weights, token-major
    acc = acts.tile([128, TT, A], F32)         # final output accumulator

    # ---------------- gating (fp32) ----------------
    with tc.tile_pool(name="gating", bufs=1) as gp, \
         tc.tile_pool(name="gstream", bufs=2) as gs, \
         tc.tile_pool(name="gtmp", bufs=4) as gt, \
         tc.tile_pool(name="psum_g", bufs=1, space="PSUM") as pg:

        gw1_sb = gp.tile([128, D // 128, GH1], F32)
        nc.sync.dma_start(out=gw1_sb, in_=gw1.rearrange("(k p) o -> p k o", p=128))
        gw2_sb = gp.tile([128, GH1 // 128, GH2], F32)
        nc.sync.dma_start(out=gw2_sb, in_=gw2.rearrange("(k p) o -> p k o", p=128))
        gw3_sb = gp.tile([128, E], F32)
        nc.sync.dma_start(out=gw3_sb, in_=gw3[:, :])
        gb1_sb = gp.tile([128, GH1 // 128], F32)
        nc.sync.dma_start(out=gb1_sb, in_=gb1[:, :])
        gb2_sb = gp.tile([128, GH2 // 128], F32)
        nc.sync.dma_start(out=gb2_sb, in_=gb2[:, :])
        g1 = gp.tile([128, GH1 // 128, T], F32)
        g2 = gp.tile([128, GH2 // 128, T], F32)

        # L1: 512 -> 256
        for n in range(NT):
            ob = gs.tile([128, D // 128, 512], F32, tag="gobs")
            nc.sync.dma_start(
                out=ob,
                in_=obs_f.rearrange("(k p) t -> p k t", p=128)[:, :, n * 512:(n + 1) * 512])
            for m in range(GH1 // 128):
                ps = pg.tile([128, 512], F32, tag="gps")
                for k in range(D // 128):
                    nc.tensor.matmul(ps, gw1_sb[:, k, m * 128:(m + 1) * 128],
                                     ob[:, k, :], start=(k == 0), stop=(k == D // 128 - 1))
                _elu_g(nc, gt, ps, gb1_sb[:, m:m + 1], g1[:, m, n * 512:(n + 1) * 512])
        # L2: 256 -> 128
        for n in range(NT):
            ps = pg.tile([128, 512], F32, tag="gps")
            for k in range(GH1 // 128):
                nc.tensor.matmul(ps, gw2_sb[:, k, :], g1[:, k, n * 512:(n + 1) * 512],
                                 start=(k == 0), stop=(k == GH1 // 128 - 1))
            _elu_g(nc, gt, ps, gb2_sb[:, 0:1], g2[:, 0, n * 512:(n + 1) * 512])
        # logits + top-2 combine weights, token-major per 128-token tile
        for t in range(TT):
            pl = pg.tile([128, E], F32, tag="gpl")
            nc.tensor.matmul(pl, g2[:, 0, t * 128:(t + 1) * 128], gw3_sb,
                             start=True, stop=False)
            nc.tensor.matmul(pl, ones_f, gb3_sb, start=False, stop=True)

            mx = gt.tile([128, 1], F32, tag="mx")
            nmx = gt.tile([128, 1], F32, tag="nmx")
            z = gt.tile([128, E], F32, tag="z")
            eq1 = gt.tile([128, E], F32, tag="eq1")
            z2 = gt.tile([128, E], F32, tag="z2")
            m2 = gt.tile([128, 1], F32, tag="m2")
            eq2 = gt.tile([128, E], F32, tag="eq2")
            msk = gt.tile([128, E], F32, tag="msk")
            num = gt.tile([128, E], F32, tag="num")
            den = gt.tile([128, 1], F32, tag="den")
            rec = gt.tile([128, 1], F32, tag="rec")

            nc.vector.reduce_max(out=mx, in_=pl, axis=X)
            nc.vector.tensor_scalar_mul(nmx, mx, -1.0)
            nc.vector.tensor_scalar(eq1, pl, mx, None, OP.is_ge)
            nc.scalar.activation(z, pl, AF.Exp, bias=nmx)
            nc.vector.tensor_sub(z2, z, eq1)
            nc.vector.reduce_max(out=m2, in_=z2, axis=X)
            nc.vector.tensor_scalar(eq2, z2, m2, None, OP.is_ge)
            nc.vector.tensor_add(msk, eq1, eq2)
            nc.vector.tensor_mul(num, z, msk)
            nc.vector.tensor_scalar_add(den, m2, 1.0)
            nc.vector.reciprocal(rec, den)
            nc.vector.tensor_scalar_mul(cw[:, t, :], num, rec)

    # ---------------- experts (bf16) ----------------
    for e in range(E):
        w1_sb = wpool.tile([128, D // 128, EH1], BF16, tag="w1")
        nc.sync.dma_start(out=w1_sb, in_=ew1[e].rearrange("(k p) o -> p k o", p=128))
        w2_sb = wpool.tile([128, EH1 // 128, EH2], BF16, tag="w2")
        nc.sync.dma_start(out=w2_sb, in_=ew2[e].rearrange("(k p) o -> p k o", p=128))
        w3_sb = wpool.tile([128, EH2 // 128, EH3], BF16, tag="w3")
        nc.sync.dma_start(out=w3_sb, in_=ew3[e].rearrange("(k p) o -> p k o", p=128))
        w4_sb = wpool.tile([128, EH3 // 128, A], BF16, tag="w4")
        nc.sync.dma_start(out=w4_sb, in_=ew4[e].rearrange("(k p) o -> p k o", p=128))
        b1_sb = wpool.tile([128, EH1 // 128], F32, tag="b1")
        nc.sync.dma_start(out=b1_sb, in_=eb1[e])
        b2_sb = wpool.tile([128, EH2 // 128], F32, tag="b2")
        nc.sync.dma_start(out=b2_sb, in_=eb2[e])
        b3_sb = wpool.tile([128, EH3 // 128], F32, tag="b3")
        nc.sync.dma_start(out=b3_sb, in_=eb3[e])

        # L1: 512 -> 1024
        for n in range(NT):
            for m in range(EH1 // 128):
                ps = psum_mm.tile([128, 512], F32, tag="mm")
                for k in range(D // 128):
                    nc.tensor.matmul(ps, w1_sb[:, k, m * 128:(m + 1) * 128],
                                     obs_sb[:, k, n * 512:(n + 1) * 512],
                                     start=(k == 0), stop=(k == D // 128 - 1))
                _elu(nc, tmp, ps, b1_sb[:, m:m + 1], h1[:, m, n * 512:(n + 1) * 512])
        # L2: 1024 -> 512
        for n in range(NT):
            for m in range(EH2 // 128):
                ps = psum_mm.tile([128, 512], F32, tag="mm")
                for k in range(EH1 // 128):
                    nc.tensor.matmul(ps, w2_sb[:, k, m * 128:(m + 1) * 128],
                                     h1[:, k, n * 512:(n + 1) * 512],
                                     start=(k == 0), stop=(k == EH1 // 128 - 1))
                _elu(nc, tmp, ps, b2_sb[:, m:m + 1], h2[:, m, n * 512:(n + 1) * 512])
        # L3: 512 -> 256
        for n in range(NT):
            for m in range(EH3 // 128):
                ps = psum_mm.tile([128, 512], F32, tag="mm")
                for k in range(EH2 // 128):
                    nc.tensor.matmul(ps, w3_sb[:, k, m * 128:(m + 1) * 128],
                                     h2[:, k, n * 512:(n + 1) * 512],
                                     start=(k == 0), stop=(k == EH2 // 128 - 1))
                _elu(nc, tmp, ps, b3_sb[:, m:m + 1], h3[:, m, n * 512:(n + 1) * 512])
        # L4 + weighted combine: 256 -> 32, token-major
        for t in range(TT):
            ps = psum_mm.tile([128, 512], F32, tag="mm")
            p4 = ps[:, :A]
            for k in range(EH3 // 128):
                nc.tensor.matmul(p4, h3[:, k, t * 128:(t + 1) * 128], w4_sb[:, k, :],
                                 start=(k == 0), stop=False)
            nc.tensor.matmul(p4, ones_f, b4_sb[:, e, :], start=False, stop=True)
            if e == 0:
                nc.vector.tensor_scalar(acc[:, t, :], p4, cw[:, t, e:e + 1], None,
                                        OP.mult)
            else:
                nc.vector.scalar_tensor_tensor(acc[:, t, :], p4, cw[:, t, e:e + 1],
                                               acc[:, t, :], OP.mult, OP.add)

    # ---------------- store ----------------
    nc.sync.dma_start(out=out.rearrange("(t p) a -> p t a", p=128), in_=acc)

    ctx.close()


def _elu_g(nc, pool, psum, bias_col, h_out):
    """fp32 ELU for the gating net."""
    t = pool.tile([128, 512], F32, tag="gelu_t")
    u = pool.tile([128, 512], F32, tag="gelu_u")
    n = psum.free_size()
    nc.scalar.activation(t[:, :n], psum, mybir.ActivationFunctionType.Exp,
                         bias=bias_col)
    nc.vector.tensor_scalar(u[:, :n], t[:, :n], -1.0, 0.0,
                            mybir.AluOpType.add, mybir.AluOpType.min)
    nc.vector.scalar_tensor_tensor(h_out, psum, bias_col, u[:, :n],
                                   mybir.AluOpType.add, mybir.AluOpType.max)


_CACHED_NC = None


def kernel(**inputs) -> np.ndarray:
    global LAST_RESULTS, _CACHED_NC
    obs = np.ascontiguousarray(inputs["observations"], dtype=np.float32)

    def pp_bias(b):  # [chunks*128] -> [128, chunks] per-partition layout
        c = b.shape[-1] // 128
        return np.ascontiguousarray(
            b.reshape(b.shape[:-1] + (c, 128)).swapaxes(-1, -2), dtype=np.float32)

    gw1 = np.asarray(inputs["gw1"], np.float32)
    gw2 = np.asarray(inputs["gw2"], np.float32)
    gw3 = np.asarray(inputs["gw3"], np.float32)
    gb1 = pp_bias(np.asarray(inputs["gb1"], np.float32))
    gb2 = pp_bias(np.asarray(inputs["gb2"], np.float32))
    gb3 = np.asarray(inputs["gb3"], np.float32).reshape(1, E)
    ew1 = np.ascontiguousarray(inputs["ew1"], dtype=np.float32).astype(NP_BF16)
    ew2 = np.ascontiguousarray(inputs["ew2"], dtype=np.float32).astype(NP_BF16)
    ew3 = np.ascontiguousarray(inputs["ew3"], dtype=np.float32).astype(NP_BF16)
    ew4 = np.ascontiguousarray(inputs["ew4"], dtype=np.float32).astype(NP_BF16)
    eb1 = pp_bias(np.asarray(inputs["eb1"], np.float32))
    eb2 = pp_bias(np.asarray(inputs["eb2"], np.float32))
    eb3 = pp_bias(np.asarray(inputs["eb3"], np.float32))
    eb4 = np.ascontiguousarray(np.asarray(inputs["eb4"], np.float32).reshape(1, E, A))

    shared = {
        "gw1": gw1, "gw2": gw2, "gw3": gw3,
        "gb1": gb1, "gb2": gb2, "gb3": gb3,
        "ew1": ew1, "ew2": ew2, "ew3": ew3, "ew4": ew4,
        "eb1": eb1, "eb2": eb2, "eb3": eb3, "eb4": eb4,
    }
    in_maps = []
    for c in range(NCORES):
        sl = obs[c * T:(c + 1) * T]                    # [T, D]
        obs_t = np.ascontiguousarray(sl.T)             # [D, T] fp32
        m = dict(shared)
        m["obs_f"] = obs_t
        m["obs_b"] = obs_t.astype(NP_BF16)
        in_maps.append(m)

    if _CACHED_NC is None:
        _CACHED_NC = _build_bass()
    nc = _CACHED_NC

    LAST_RESULTS = run_bass_kernel_spmd(nc, in_maps, core_ids=list(range(NCORES)))
    outs = [LAST_RESULTS.results[c]["out"] for c in range(NCORES)]
    return np.concatenate(outs, axis=0).astype(np.float32)


# revision 19
# speedup vs baseline: 1.0606x; 1.0038x over previous
"""MoE actor-critic forward kernel for 8 Trainium2 NeuronCores.

Strategy: data-parallel over the batch axis. Each of the 8 cores gets
B/8 = 2048 tokens plus a full replica of the gating + expert weights.

Per-core math (all activations feature-major [feat_part, tok] in SBUF):
  - gating MLP in fp32 (exact routing): 512 -> 256 -> 128 -> 8 logits
    token-major, then top-2 + renormalized combine weights cw[tok, 8]
    computed with exp/compare/reduce ops (no explicit softmax division
    needed: cw = z * (eq1+eq2) / (1 + m2) with z = exp(l - lmax)).
  - experts in bf16 (fp32 PSUM accumulate): all 8 experts run densely,
    512 -> 1024 -> 512 -> 256 -> 32 with ELU between layers.
    ELU(x) = max(x + b, min(exp(x + b) - 1, 0)) -> 1 ACT + 2 DVE ops.
  - combine: acc[tok, 32] += cw[:, e] * (h3_e @ W4_e + b4_e), where the
    bias enters the matmul via a K=1 ones-row matmul.
"""

import numpy as np
import ml_dtypes

import concourse.bass as bass
import concourse.mybir as mybir
import concourse.tile as tile
from concourse import bacc
from concourse.bass_utils import run_bass_kernel_spmd

BF16 = mybir.dt.bfloat16
F32 = mybir.dt.float32
NP_BF16 = ml_dtypes.bfloat16

B = 16384
D = 512          # obs dim
A = 32           # actions
E = 8            # experts
NCORES = 8
T = B // NCORES  # tokens per core (2048)
NT = T // 512    # 512-token tiles (4)
TT = T // 128    # 128-token tiles (16)

EH1, EH2, EH3 = 1024, 512, 256
GH1, GH2 = 256, 128

LAST_RESULTS = None  # test harness reads exec_time_ns from here


def _build_bass():
    nc = bacc.Bacc("TRN2", target_bir_lowering=False, debug=False,
                   enable_asserts=False, num_devices=NCORES)

    # ---- DRAM I/O ----
    obs_f = nc.dram_tensor("obs_f", [D, T], F32, kind="ExternalInput")
    obs_b = nc.dram_tensor("obs_b", [D, T], BF16, kind="ExternalInput")
    gw1 = nc.dram_tensor("gw1", [D, GH1], F32, kind="ExternalInput")
    gw2 = nc.dram_tensor("gw2", [GH1, GH2], F32, kind="ExternalInput")
    gw3 = nc.dram_tensor("gw3", [GH2, E], F32, kind="ExternalInput")
    gb1 = nc.dram_tensor("gb1", [128, GH1 // 128], F32, kind="ExternalInput")
    gb2 = nc.dram_tensor("gb2", [128, GH2 // 128], F32, kind="ExternalInput")
    gb3 = nc.dram_tensor("gb3", [1, E], F32, kind="ExternalInput")
    ew1 = nc.dram_tensor("ew1", [E, D, EH1], BF16, kind="ExternalInput")
    ew2 = nc.dram_tensor("ew2", [E, EH1, EH2], BF16, kind="ExternalInput")
    ew3 = nc.dram_tensor("ew3", [E, EH2, EH3], BF16, kind="ExternalInput")
    ew4 = nc.dram_tensor("ew4", [E, EH3, A], BF16, kind="ExternalInput")
    eb1 = nc.dram_tensor("eb1", [E, 128, EH1 // 128], F32, kind="ExternalInput")
    eb2 = nc.dram_tensor("eb2", [E, 128, EH2 // 128], F32, kind="ExternalInput")
    eb3 = nc.dram_tensor("eb3", [E, 128, EH3 // 128], F32, kind="ExternalInput")
    eb4 = nc.dram_tensor("eb4", [1, E, A], F32, kind="ExternalInput")
    out = nc.dram_tensor("out", [T, A], F32, kind="ExternalOutput")

    with tile.TileContext(nc) as tc:
        _emit(nc, tc, obs_f, obs_b, gw1, gw2, gw3, gb1, gb2, gb3,
              ew1, ew2, ew3, ew4, eb1, eb2, eb3, eb4, out)
    nc.compile()
    return nc


def _elu(nc, pool, psum, bias_col, h_out):
    """h_out = ELU(psum + bias_col) = max(x+b, min(exp(x+b)-1, 0))."""
    p, n = psum.shape[0], psum.free_size()
    t = pool.tile([128, 512], BF16, tag="elu_t")
    u = pool.tile([128, 512], BF16, tag="elu_u")
    nc.scalar.activation(t[:p, :n], psum, mybir.ActivationFunctionType.Exp,
                         bias=bias_col)
    nc.vector.tensor_scalar(u[:p, :n], t[:p, :n], -1.0, 0.0,
                            mybir.AluOpType.add, mybir.AluOpType.min)
    nc.vector.scalar_tensor_tensor(h_out, psum, bias_col, u[:p, :n],
                                   mybir.AluOpType.add, mybir.AluOpType.max)


def _emit(nc, tc, obs_f, obs_b, gw1, gw2, gw3, gb1, gb2, gb3,
          ew1, ew2, ew3, ew4, eb1, eb2, eb3, eb4, out):
    AF = mybir.ActivationFunctionType
    OP = mybir.AluOpType
    X = mybir.AxisListType.X

    # ---------------- persistent pools ----------------
    from contextlib import ExitStack
    ctx = ExitStack()
    consts = ctx.enter_context(tc.tile_pool(name="consts", bufs=1))
    acts = ctx.enter_context(tc.tile_pool(name="acts", bufs=1))
    wpool = ctx.enter_context(tc.tile_pool(name="wpool", bufs=2))
    tmp = ctx.enter_context(tc.tile_pool(name="tmp", bufs=4))
    psum_mm = ctx.enter_context(tc.tile_pool(name="psum_mm", bufs=6, space="PSUM"))

    # constants
    ones_b = consts.tile([1, 128], BF16)
    nc.vector.memset(ones_b, 1.0)
    ones_f = consts.tile([1, 128], F32)
    nc.vector.memset(ones_f, 1.0)
    b4_sb = consts.tile([1, E, A], F32)
    nc.sync.dma_start(out=b4_sb, in_=eb4[:, :, :])
    gb3_sb = consts.tile([1, E], F32)
    nc.sync.dma_start(out=gb3_sb, in_=gb3[:, :])

    # persistent activations (feature-major, bf16)
    obs_sb = acts.tile([128, D // 128, T], BF16)
    nc.sync.dma_start(out=obs_sb, in_=obs_b.rearrange("(k p) t -> p k t", p=128))
    h1 = acts.tile([128, EH1 // 128, T], BF16)
    h2 = acts.tile([128, EH2 // 128, T], BF16)
    h3 = acts.tile([128, EH3 // 128, T], BF16)
    cw = acts.tile([128, TT, E], F32)          # combine weights, token-major
    acc = acts.tile([128, TT, A], F32)         # final output accumulator

    # ---------------- gating (fp32) ----------------
    with tc.tile_pool(name="gating", bufs=1) as gp, \
         tc.tile_pool(name="gstream", bufs=2) as gs, \
         tc.tile_pool(name="gtmp", bufs=4) as gt, \
         tc.tile_pool(name="psum_g", bufs=1, space="PSUM") as pg:

        gw1_sb = gp.tile([128, D // 128, GH1], F32)
        nc.sync.dma_start(out=gw1_sb, in_=gw1.rearrange("(k p) o -> p k o", p=128))
        gw2_sb = gp.tile([128, GH1 // 128, GH2], F32)
        nc.sync.dma_start(out=gw2_sb, in_=gw2.rearrange("(k p) o -> p k o", p=128))
        gw3_sb = gp.tile([128, E], F32)
        nc.sync.dma_start(out=gw3_sb, in_=gw3[:, :])
        gb1_sb = gp.tile([128, GH1 // 128], F32)
        nc.sync.dma_start(out=gb1_sb, in_=gb1[:, :])
        gb2_sb = gp.tile([128, GH2 // 128], F32)
        nc.sync.dma_start(out=gb2_sb, in_=gb2[:, :])
        g1 = gp.tile([128, GH1 // 128, T], F32)
        g2 = gp.tile([128, GH2 // 128, T], F32)

        # L1: 512 -> 256
        for n in range(NT):
            ob = gs.tile([128, D // 128, 512], F32, tag="gobs")
            nc.sync.dma_start(
                out=ob,
                in_=obs_f.rearrange("(k p) t -> p k t", p=128)[:, :, n * 512:(n + 1) * 512])
            for m in range(GH1 // 128):
                ps = pg.tile([128, 512], F32, tag="gps")
                for k in range(D // 128):
                    nc.tensor.matmul(ps, gw1_sb[:, k, m * 128:(m + 1) * 128],
                                     ob[:, k, :], start=(k == 0), stop=(k == D // 128 - 1))
                _elu_g(nc, gt, ps, gb1_sb[:, m:m + 1], g1[:, m, n * 512:(n + 1) * 512])
        # L2: 256 -> 128
        for n in range(NT):
            ps = pg.tile([128, 512], F32, tag="gps")
            for k in range(GH1 // 128):
                nc.tensor.matmul(ps, gw2_sb[:, k, :], g1[:, k, n * 512:(n + 1) * 512],
                                 start=(k == 0), stop=(k == GH1 // 128 - 1))
            _elu_g(nc, gt, ps, gb2_sb[:, 0:1], g2[:, 0, n * 512:(n + 1) * 512])
        # logits + top-2 combine weights, token-major per 128-token tile
        for t in range(TT):
            pl = pg.tile([128, E], F32, tag="gpl")
            nc.tensor.matmul(pl, g2[:, 0, t * 128:(t + 1) * 128], gw3_sb,
                             start=True, stop=False)
            nc.tensor.matmul(pl, ones_f, gb3_sb, start=False, stop=True)

            mx = gt.tile([128, 1], F32, tag="mx")
            nmx = gt.tile([128, 1], F32, tag="nmx")
            z = gt.tile([128, E], F32, tag="z")
            eq1 = gt.tile([128, E], F32, tag="eq1")
            z2 = gt.tile([128, E], F32, tag="z2")
            m2 = gt.tile([128, 1], F32, tag="m2")
            eq2 = gt.tile([128, E], F32, tag="eq2")
            msk = gt.tile([128, E], F32, tag="msk")
            num = gt.tile([128, E], F32, tag="num")
            den = gt.tile([128, 1], F32, tag="den")
            rec = gt.tile([128, 1], F32, tag="rec")

            nc.vector.reduce_max(out=mx, in_=pl, axis=X)
            nc.vector.tensor_scalar_mul(nmx, mx, -1.0)
            nc.vector.tensor_scalar(eq1, pl, mx, None, OP.is_ge)
            nc.scalar.activation(z, pl, AF.Exp, bias=nmx)
            nc.vector.tensor_sub(z2, z, eq1)
            nc.vector.reduce_max(out=m2, in_=z2, axis=X)
            nc.vector.tensor_scalar(eq2, z2, m2, None, OP.is_ge)
            nc.vector.tensor_add(msk, eq1, eq2)
            nc.vector.tensor_mul(num, z, msk)
            nc.vector.tensor_scalar_add(den, m2, 1.0)
            nc.vector.reciprocal(rec, den)
            nc.vector.tensor_scalar_mul(cw[:, t, :], num, rec)

    # ---------------- experts (bf16) ----------------
    for e in range(E):
        w1_sb = wpool.tile([128, D // 128, EH1], BF16, tag="w1")
        nc.sync.dma_start(out=w1_sb, in_=ew1[e].rearrange("(k p) o -> p k o", p=128))
        w2_sb = wpool.tile([128, EH1 // 128, EH2], BF16, tag="w2")
        nc.sync.dma_start(out=w2_sb, in_=ew2[e].rearrange("(k p) o -> p k o", p=128))
        w3_sb = wpool.tile([128, EH2 // 128, EH3], BF16, tag="w3")
        nc.sync.dma_start(out=w3_sb, in_=ew3[e].rearrange("(k p) o -> p k o", p=128))
        w4_sb = wpool.tile([128, EH3 // 128, A], BF16, tag="w4")
        nc.sync.dma_start(out=w4_sb, in_=ew4[e].rearrange("(k p) o -> p k o", p=128))
        b1_sb = wpool.tile([128, EH1 // 128], F32, tag="b1")
        nc.sync.dma_start(out=b1_sb, in_=eb1[e])
        b2_sb = wpool.tile([128, EH2 // 128], F32, tag="b2")
        nc.sync.dma_start(out=b2_sb, in_=eb2[e])
        b3_sb = wpool.tile([128, EH3 // 128], F32, tag="b3")
        nc.sync.dma_start(out=b3_sb, in_=eb3[e])

        # L1: 512 -> 1024
        for n in range(NT):
            for m in range(EH1 // 128):
                ps = psum_mm.tile([128, 512], F32, tag="mm")
                for k in range(D // 128):
                    nc.tensor.matmul(ps, w1_sb[:, k, m * 128:(m + 1) * 128],
                                     obs_sb[:, k, n * 512:(n + 1) * 512],
                                     start=(k == 0), stop=(k == D // 128 - 1))
                _elu(nc, tmp, ps, b1_sb[:, m:m + 1], h1[:, m, n * 512:(n + 1) * 512])
        # L2: 1024 -> 512
        for n in range(NT):
            for m in range(EH2 // 128):
                ps = psum_mm.tile([128, 512], F32, tag="mm")
                for k in range(EH1 // 128):
                    nc.tensor.matmul(ps, w2_sb[:, k, m * 128:(m + 1) * 128],
                                     h1[:, k, n * 512:(n + 1) * 512],
                                     start=(k == 0), stop=(k == EH1 // 128 - 1))
                _elu(nc, tmp, ps, b2_sb[:, m:m + 1], h2[:, m, n * 512:(n + 1) * 512])
        # L3: 512 -> 256
        for n in range(NT):
            for m in range(EH3 // 128):
                ps = psum_mm.tile([128, 512], F32, tag="mm")
                for k in range(EH2 // 128):
                    nc.tensor.matmul(ps, w3_sb[:, k, m * 128:(m + 1) * 128],
                                     h2[:, k, n * 512:(n + 1) * 512],
                                     start=(k == 0), stop=(k == EH2 // 128 - 1))
                _elu(nc, tmp, ps, b3_sb[:, m:m + 1], h3[:, m, n * 512:(n + 1) * 512])
        # L4 + weighted combine: 256 -> 32, token-major
        for t in range(TT):
            ps = psum_mm.tile([128, 512], F32, tag="mm")
            p4 = ps[:, :A]
            for k in range(EH3 // 128):
                nc.tensor.matmul(p4, h3[:, k, t * 128:(t + 1) * 128], w4_sb[:, k, :],
                                 start=(k == 0), stop=False)
            nc.tensor.matmul(p4, ones_f, b4_sb[:, e, :], start=False, stop=True)
            if e == 0:
                nc.vector.tensor_scalar(acc[:, t, :], p4, cw[:, t, e:e + 1], None,
                                        OP.mult)
            else:
                nc.vector.scalar_tensor_tensor(acc[:, t, :], p4, cw[:, t, e:e + 1],
                                               acc[:, t, :], OP.mult, OP.add)

    # ---------------- store ----------------
    nc.sync.dma_start(out=out.rearrange("(t p) a -> p t a", p=128), in_=acc)

    ctx.close()


def _elu_g(nc, pool, psum, bias_col, h_out):
    """fp32 ELU for the gating net."""
    t = pool.tile([128, 512], F32, tag="gelu_t")
    u = pool.tile([128, 512], F32, tag="gelu_u")
    n = psum.free_size()
    nc.scalar.activation(t[:, :n], psum, mybir.ActivationFunctionType.Exp,
                         bias=bias_col)
    nc.vector.tensor_scalar(u[:, :n], t[:, :n], -1.0, 0.0,
                            mybir.AluOpType.add, mybir.AluOpType.min)
    nc.vector.scalar_tensor_tensor(h_out, psum, bias_col, u[:, :n],
                                   mybir.AluOpType.add, mybir.AluOpType.max)


_CACHED_NC = None


def kernel(**inputs) -> np.ndarray:
    global LAST_RESULTS, _CACHED_NC
    obs = np.ascontiguousarray(inputs["observations"], dtype=np.float32)

    def pp_bias(b):  # [chunks*128] -> [128, chunks] per-partition layout
        c = b.shape[-1] // 128
        return np.ascontiguousarray(
            b.reshape(b.shape[:-1] + (c, 128)).swapaxes(-1, -2), dtype=np.float32)

    gw1 = np.asarray(inputs["gw1"], np.float32)
    gw2 = np.asarray(inputs["gw2"], np.float32)
    gw3 = np.asarray(inputs["gw3"], np.float32)
    gb1 = pp_bias(np.asarray(inputs["gb1"], np.float32))
    gb2 = pp_bias(np.asarray(inputs["gb2"], np.float32))
    gb3 = np.asarray(inputs["gb3"], np.float32).reshape(1, E)
    ew1 = np.ascontiguousarray(inputs["ew1"], dtype=np.float32).astype(NP_BF16)
    ew2 = np.ascontiguousarray(inputs["ew2"], dtype=np.float32).astype(NP_BF16)
    ew3 = np.ascontiguousarray(inputs["ew3"], dtype=np.float32).astype(NP_BF16)
    ew4 = np.ascontiguousarray(inputs["ew4"], dtype=np.float32).astype(NP_BF16)
    eb1 = pp_bias(np.asarray(inputs["eb1"], np.float32))
    eb2 = pp_bias(np.asarray(inputs["eb2"], np.float32))
    eb3 = pp_bias(np.asarray(inputs["eb3"], np.float32))
    eb4 = np.ascontiguousarray(np.asarray(inputs["eb4"], np.float32).reshape(1, E, A))

    shared = {
        "gw1": gw1, "gw2": gw2, "gw3": gw3,
        "gb1": gb1, "gb2": gb2, "gb3": gb3,
        "ew1": ew1, "ew2": ew2, "ew3": ew3, "ew4": ew4,
        "eb1": eb1, "eb2": eb2, "eb3": eb3, "eb4": eb4,
    }
    in_maps = []
    for c in range(NCORES):
        sl = obs[c * T:(c + 1) * T]                    # [T, D]
        obs_t = np.ascontiguousarray(sl.T)             # [D, T] fp32
        m = dict(shared)
        m["obs_f"] = obs_t
        m["obs_b"] = obs_t.astype(NP_BF16)
        in_maps.append(m)

    if _CACHED_NC is None:
        _CACHED_NC = _build_bass()
    nc = _CACHED_NC

    LAST_RESULTS = run_bass_kernel_spmd(nc, in_maps, core_ids=list(range(NCORES)))
    outs = [LAST_RESULTS.results[c]["out"] for c in range(NCORES)]
    return np.concatenate(outs, axis=0).astype(np.float32)


# revision 43
# speedup vs baseline: 1.0818x; 1.0200x over previous
"""MoE actor-critic forward kernel for 8 Trainium2 NeuronCores.

Strategy: data-parallel over the batch axis. Each of the 8 cores gets
B/8 = 2048 tokens plus a full replica of the gating + expert weights.

Per-core math (all activations feature-major [feat_part, tok] in SBUF):
  - gating MLP in fp32 (exact routing): 512 -> 256 -> 128 -> 8 logits
    token-major, then top-2 + renormalized combine weights cw[tok, 8]
    computed with exp/compare/reduce ops (no explicit softmax division
    needed: cw = z * (eq1+eq2) / (1 + m2) with z = exp(l - lmax)).
  - experts in bf16 (fp32 PSUM accumulate): all 8 experts run densely,
    512 -> 1024 -> 512 -> 256 -> 32 with ELU between layers.
    ELU(x) = max(x + b, min(exp(x + b) - 1, 0)) -> 1 ACT + 2 DVE ops.
  - combine: acc[tok, 32] += cw[:, e] * (h3_e @ W4_e + b4_e), where the
    bias enters the matmul via a K=1 ones-row matmul.
"""

import numpy as np
import ml_dtypes

import concourse.bass as bass
import concourse.mybir as mybir
import concourse.tile as tile
from concourse import bacc
from concourse.bass_utils import run_bass_kernel_spmd

BF16 = mybir.dt.bfloat16
F32 = mybir.dt.float32
NP_BF16 = ml_dtypes.bfloat16

B = 16384
D = 512          # obs dim
A = 32           # actions
E = 8            # experts
NCORES = 8
T = B // NCORES  # tokens per core (2048)
NT = T // 512    # 512-token tiles (4)
TT = T // 128    # 128-token tiles (16)

EH1, EH2, EH3 = 1024, 512, 256
GH1, GH2 = 256, 128

LAST_RESULTS = None  # test harness reads exec_time_ns from here


def _build_bass():
    nc = bacc.Bacc("TRN2", target_bir_lowering=False, debug=False,
                   enable_asserts=False, num_devices=NCORES)

    # ---- DRAM I/O ----
    obs_f = nc.dram_tensor("obs_f", [D, T], F32, kind="ExternalInput")
    obs_b = nc.dram_tensor("obs_b", [D, T], BF16, kind="ExternalInput")
    gw1 = nc.dram_tensor("gw1", [D, GH1], F32, kind="ExternalInput")
    gw2 = nc.dram_tensor("gw2", [GH1, GH2], F32, kind="ExternalInput")
    gw3 = nc.dram_tensor("gw3", [GH2, E], F32, kind="ExternalInput")
    gb1 = nc.dram_tensor("gb1", [128, GH1 // 128], F32, kind="ExternalInput")
    gb2 = nc.dram_tensor("gb2", [128, GH2 // 128], F32, kind="ExternalInput")
    gb3 = nc.dram_tensor("gb3", [1, E], F32, kind="ExternalInput")
    ew1 = nc.dram_tensor("ew1", [E, D, EH1], BF16, kind="ExternalInput")
    ew2 = nc.dram_tensor("ew2", [E, EH1, EH2], BF16, kind="ExternalInput")
    ew3 = nc.dram_tensor("ew3", [E, EH2, EH3], BF16, kind="ExternalInput")
    ew4 = nc.dram_tensor("ew4", [E, EH3, A], BF16, kind="ExternalInput")
    eb1 = nc.dram_tensor("eb1", [E, 128, EH1 // 128], F32, kind="ExternalInput")
    eb2 = nc.dram_tensor("eb2", [E, 128, EH2 // 128], F32, kind="ExternalInput")
    eb3 = nc.dram_tensor("eb3", [E, 128, EH3 // 128], F32, kind="ExternalInput")
    eb4 = nc.dram_tensor("eb4", [1, E, A], F32, kind="ExternalInput")
    out = nc.dram_tensor("out", [T, A], F32, kind="ExternalOutput")

    with tile.TileContext(nc) as tc:
        _emit(nc, tc, obs_f, obs_b, gw1, gw2, gw3, gb1, gb2, gb3,
              ew1, ew2, ew3, ew4, eb1, eb2, eb3, eb4, out)
    nc.compile()
    return nc


def _elu(nc, pool, psum, bias_col, h_out):
    """h_out = ELU(psum + bias_col) = max(x+b, min(exp(x+b)-1, 0))."""
    p, n = psum.shape[0], psum.free_size()
    t = pool.tile([128, 512], BF16, tag="elu_t")
    u = pool.tile([128, 512], BF16, tag="elu_u")
    nc.scalar.activation(t[:p, :n], psum, mybir.ActivationFunctionType.Exp,
                         bias=bias_col)
    nc.vector.tensor_scalar(u[:p, :n], t[:p, :n], -1.0, 0.0,
                            mybir.AluOpType.add, mybir.AluOpType.min)
    nc.vector.scalar_tensor_tensor(h_out, psum, bias_col, u[:p, :n],
                                   mybir.AluOpType.add, mybir.AluOpType.max)


def _emit(nc, tc, obs_f, obs_b, gw1, gw2, gw3, gb1, gb2, gb3,
          ew1, ew2, ew3, ew4, eb1, eb2, eb3, eb4, out):
    AF = mybir.ActivationFunctionType
    OP = mybir.AluOpType
    X = mybir.AxisListType.X

    # ---------------- persistent pools ----------------
    from contextlib import ExitStack
    ctx = ExitStack()
    consts = ctx.enter_context(tc.tile_pool(name="consts", bufs=1))
    acts = ctx.enter_context(tc.tile_pool(name="acts", bufs=1))
    wpool = ctx.enter_context(tc.tile_pool(name="wpool", bufs=2))
    tmp = ctx.enter_context(tc.tile_pool(name="tmp", bufs=4))
    psum_mm = ctx.enter_context(tc.tile_pool(name="psum_mm", bufs=6, space="PSUM"))

    # constants
    ones_b = consts.tile([1, 128], BF16)
    nc.vector.memset(ones_b, 1.0)
    ones_f = consts.tile([1, 128], F32)
    nc.vector.memset(ones_f, 1.0)
    b4_sb = consts.tile([1, E, A], F32)
    nc.sync.dma_start(out=b4_sb, in_=eb4[:, :, :])
    gb3_sb = consts.tile([1, E], F32)
    nc.sync.dma_start(out=gb3_sb, in_=gb3[:, :])

    # persistent activations (feature-major, bf16)
    obs_sb = acts.tile([128, D // 128, T], BF16)
    nc.sync.dma_start(out=obs_sb, in_=obs_b.rearrange("(k p) t -> p k t", p=128))
    h1 = acts.tile([128, EH1 // 128, T], BF16)
    h2 = acts.tile([128, EH2 // 128, T], BF16)
    h3 = acts.tile([128, EH3 // 128, T], BF16)
    cw = acts.tile([128, TT, E], F32)          # combine weights, token-major
    # per-token-tile accumulators (separate tiles so the 16 combine chains
    # don't serialize on one tile's write-write deps)
    acc = [acts.tile([128, A], F32, tag=f"acc{t}", name=f"acc{t}")
           for t in range(TT)]

    # ---------------- gating (fp32) ----------------
    with tc.tile_pool(name="gating", bufs=1) as gp, \
         tc.tile_pool(name="gstream", bufs=2) as gs, \
         tc.tile_pool(name="gtmp", bufs=4) as gt, \
         tc.tile_pool(name="psum_g", bufs=1, space="PSUM") as pg:

        gw1_sb = gp.tile([128, D // 128, GH1], F32)
        for k in range(D // 128):
            nc.sync.dma_start(out=gw1_sb[:, k, :],
                              in_=gw1.rearrange("(k p) o -> p k o", p=128)[:, k, :])
        gw2_sb = gp.tile([128, GH1 // 128, GH2], F32)
        nc.sync.dma_start(out=gw2_sb, in_=gw2.rearrange("(k p) o -> p k o", p=128))
        gw3_sb = gp.tile([128, E], F32)
        nc.sync.dma_start(out=gw3_sb, in_=gw3[:, :])
        gb1_sb = gp.tile([128, GH1 // 128], F32)
        nc.sync.dma_start(out=gb1_sb, in_=gb1[:, :])
        gb2_sb = gp.tile([128, GH2 // 128], F32)
        nc.sync.dma_start(out=gb2_sb, in_=gb2[:, :])
        g1 = gp.tile([128, GH1 // 128, T], F32)
        g2 = gp.tile([128, GH2 // 128, T], F32)

        # L1: 512 -> 256 (inputs streamed per K-chunk so the first matmul
        # only waits on one 256KB DMA, not the whole block)
        for n in range(NT):
            obk = []
            for k in range(D // 128):
                ob = gs.tile([128, 512], F32, tag="gobs", bufs=10,
                             name=f"ob{n}_{k}")
                nc.sync.dma_start(
                    out=ob,
                    in_=obs_f.rearrange("(k p) t -> p k t", p=128)[:, k, n * 512:(n + 1) * 512])
                obk.append(ob)
            for m in range(GH1 // 128):
                ps = pg.tile([128, 512], F32, tag="gps")
                for k in range(D // 128):
                    nc.tensor.matmul(ps, gw1_sb[:, k, m * 128:(m + 1) * 128],
                                     obk[k], start=(k == 0), stop=(k == D // 128 - 1))
                _elu_g(nc, gt, ps, gb1_sb[:, m:m + 1], g1[:, m, n * 512:(n + 1) * 512])
        # L2: 256 -> 128
        for n in range(NT):
            ps = pg.tile([128, 512], F32, tag="gps")
            for k in range(GH1 // 128):
                nc.tensor.matmul(ps, gw2_sb[:, k, :], g1[:, k, n * 512:(n + 1) * 512],
                                 start=(k == 0), stop=(k == GH1 // 128 - 1))
            _elu_g(nc, gt, ps, gb2_sb[:, 0:1], g2[:, 0, n * 512:(n + 1) * 512])
        # logits + top-2 combine weights, token-major per 128-token tile
        for t in range(TT):
            pl = pg.tile([128, E], F32, tag="gpl")
            nc.tensor.matmul(pl, g2[:, 0, t * 128:(t + 1) * 128], gw3_sb,
                             start=True, stop=False)
            nc.tensor.matmul(pl, ones_f, gb3_sb, start=False, stop=True)

            mx = gt.tile([128, 1], F32, tag="mx")
            nmx = gt.tile([128, 1], F32, tag="nmx")
            z = gt.tile([128, E], F32, tag="z")
            eq1 = gt.tile([128, E], F32, tag="eq1")
            z2 = gt.tile([128, E], F32, tag="z2")
            m2 = gt.tile([128, 1], F32, tag="m2")
            eq2 = gt.tile([128, E], F32, tag="eq2")
            msk = gt.tile([128, E], F32, tag="msk")
            num = gt.tile([128, E], F32, tag="num")
            den = gt.tile([128, 1], F32, tag="den")
            rec = gt.tile([128, 1], F32, tag="rec")

            nc.vector.reduce_max(out=mx, in_=pl, axis=X)
            nc.vector.tensor_scalar_mul(nmx, mx, -1.0)
            nc.vector.tensor_scalar(eq1, pl, mx, None, OP.is_ge)
            nc.scalar.activation(z, pl, AF.Exp, bias=nmx)
            nc.vector.tensor_sub(z2, z, eq1)
            nc.vector.reduce_max(out=m2, in_=z2, axis=X)
            nc.vector.tensor_scalar(eq2, z2, m2, None, OP.is_ge)
            nc.vector.tensor_add(msk, eq1, eq2)
            nc.vector.tensor_mul(num, z, msk)
            nc.vector.tensor_scalar_add(den, m2, 1.0)
            nc.vector.reciprocal(rec, den)
            nc.vector.tensor_scalar_mul(cw[:, t, :], num, rec)

    # L4 psums get their own pool (reusing the gating pool's PSUM banks) so
    # the next expert's L1 psums don't contend with pending combine reads
    psum_l4 = ctx.enter_context(tc.tile_pool(name="psum_l4", bufs=2, space="PSUM"))

    # ---------------- experts (bf16) ----------------
    for e in range(E):
        w1_sb = wpool.tile([128, D // 128, EH1], BF16, tag="w1")
        nc.sync.dma_start(out=w1_sb, in_=ew1[e].rearrange("(k p) o -> p k o", p=128))
        w2_sb = wpool.tile([128, EH1 // 128, EH2], BF16, tag="w2")
        nc.sync.dma_start(out=w2_sb, in_=ew2[e].rearrange("(k p) o -> p k o", p=128))
        w3_sb = wpool.tile([128, EH2 // 128, EH3], BF16, tag="w3")
        nc.sync.dma_start(out=w3_sb, in_=ew3[e].rearrange("(k p) o -> p k o", p=128))
        w4_sb = wpool.tile([128, EH3 // 128, A], BF16, tag="w4")
        nc.sync.dma_start(out=w4_sb, in_=ew4[e].rearrange("(k p) o -> p k o", p=128))
        b1_sb = wpool.tile([128, EH1 // 128], F32, tag="b1")
        nc.sync.dma_start(out=b1_sb, in_=eb1[e])
        b2_sb = wpool.tile([128, EH2 // 128], F32, tag="b2")
        nc.sync.dma_start(out=b2_sb, in_=eb2[e])
        b3_sb = wpool.tile([128, EH3 // 128], F32, tag="b3")
        nc.sync.dma_start(out=b3_sb, in_=eb3[e])

        # L1: 512 -> 1024
        for n in range(NT):
            for m in range(EH1 // 128):
                ps = psum_mm.tile([128, 512], F32, tag="mm")
                for k in range(D // 128):
                    nc.tensor.matmul(ps, w1_sb[:, k, m * 128:(m + 1) * 128],
                                     obs_sb[:, k, n * 512:(n + 1) * 512],
                                     start=(k == 0), stop=(k == D // 128 - 1))
                _elu(nc, tmp, ps, b1_sb[:, m:m + 1], h1[:, m, n * 512:(n + 1) * 512])
        # L2: 1024 -> 512
        for n in range(NT):
            for m in range(EH2 // 128):
                ps = psum_mm.tile([128, 512], F32, tag="mm")
                for k in range(EH1 // 128):
                    nc.tensor.matmul(ps, w2_sb[:, k, m * 128:(m + 1) * 128],
                                     h1[:, k, n * 512:(n + 1) * 512],
                                     start=(k == 0), stop=(k == EH1 // 128 - 1))
                _elu(nc, tmp, ps, b2_sb[:, m:m + 1], h2[:, m, n * 512:(n + 1) * 512])
        # L3: 512 -> 256
        for n in range(NT):
            for m in range(EH3 // 128):
                ps = psum_mm.tile([128, 512], F32, tag="mm")
                for k in range(EH2 // 128):
                    nc.tensor.matmul(ps, w3_sb[:, k, m * 128:(m + 1) * 128],
                                     h2[:, k, n * 512:(n + 1) * 512],
                                     start=(k == 0), stop=(k == EH2 // 128 - 1))
                _elu(nc, tmp, ps, b3_sb[:, m:m + 1], h3[:, m, n * 512:(n + 1) * 512])
        # L4 + weighted combine: 256 -> 32, token-major
        for t in range(TT):
            p4 = psum_l4.tile([128, A], F32, tag="l4")
            for k in range(EH3 // 128):
                nc.tensor.matmul(p4, h3[:, k, t * 128:(t + 1) * 128], w4_sb[:, k, :],
                                 start=(k == 0), stop=False)
            nc.tensor.matmul(p4, ones_f, b4_sb[:, e, :], start=False, stop=True)
            if e == 0:
                nc.vector.tensor_scalar(acc[t], p4, cw[:, t, e:e + 1], None,
                                        OP.mult)
            else:
                nc.vector.scalar_tensor_tensor(acc[t], p4, cw[:, t, e:e + 1],
                                               acc[t], OP.mult, OP.add)

    # ---------------- store ----------------
    for t in range(TT):
        nc.sync.dma_start(
            out=out.rearrange("(t p) a -> p t a", p=128)[:, t, :], in_=acc[t])

    ctx.close()


def _elu_g(nc, pool, psum, bias_col, h_out):
    """fp32 ELU for the gating net."""
    t = pool.tile([128, 512], F32, tag="gelu_t")
    u = pool.tile([128, 512], F32, tag="gelu_u")
    n = psum.free_size()
    nc.scalar.activation(t[:, :n], psum, mybir.ActivationFunctionType.Exp,
                         bias=bias_col)
    nc.vector.tensor_scalar(u[:, :n], t[:, :n], -1.0, 0.0,
                            mybir.AluOpType.add, mybir.AluOpType.min)
    nc.vector.scalar_tensor_tensor(h_out, psum, bias_col, u[:, :n],
                                   mybir.AluOpType.add, mybir.AluOpType.max)


_CACHED_NC = None


def kernel(**inputs) -> np.ndarray:
    global LAST_RESULTS, _CACHED_NC
    obs = np.ascontiguousarray(inputs["observations"], dtype=np.float32)

    def pp_bias(b):  # [chunks*128] -> [128, chunks] per-partition layout
        c = b.shape[-1] // 128
        return np.ascontiguousarray(
            b.reshape(b.shape[:-1] + (c, 128)).swapaxes(-1, -2), dtype=np.float32)

    gw1 = np.asarray(inputs["gw1"], np.float32)
    gw2 = np.asarray(inputs["gw2"], np.float32)
    gw3 = np.asarray(inputs["gw3"], np.float32)
    gb1 = pp_bias(np.asarray(inputs["gb1"], np.float32))
    gb2 = pp_bias(np.asarray(inputs["gb2"], np.float32))
    gb3 = np.asarray(inputs["gb3"], np.float32).reshape(1, E)
    ew1 = np.ascontiguousarray(inputs["ew1"], dtype=np.float32).astype(NP_BF16)
    ew2 = np.ascontiguousarray(inputs["ew2"], dtype=np.float32).astype(NP_BF16)
    ew3 = np.ascontiguousarray(inputs["ew3"], dtype=np.float32).astype(NP_BF16)
    ew4 = np.ascontiguousarray(inputs["ew4"], dtype=np.float32).astype(NP_BF16)
    eb1 = pp_bias(np.asarray(inputs["eb1"], np.float32))
    eb2 = pp_bias(np.asarray(inputs["eb2"], np.float32))
    eb3 = pp_bias(np.asarray(inputs["eb3"], np.float32))
    eb4 = np.ascontiguousarray(np.asarray(inputs["eb4"], np.float32).reshape(1, E, A))

    shared = {
        "gw1": gw1, "gw2": gw2, "gw3": gw3,
        "gb1": gb1, "gb2": gb2, "gb3": gb3,
        "ew1": ew1, "ew2": ew2, "ew3": ew3, "ew4": ew4,
        "eb1": eb1, "eb2": eb2, "eb3": eb3, "eb4": eb4,
    }
    in_maps = []
    for c in range(NCORES):
        sl = obs[c * T:(c + 1) * T]                    # [T, D]
        obs_t = np.ascontiguousarray(sl.T)             # [D, T] fp32
        m = dict(shared)
        m["obs_f"] = obs_t
        m["obs_b"] = obs_t.astype(NP_BF16)
        in_maps.append(m)

    if _CACHED_NC is None:
        _CACHED_NC = _build_bass()
    nc = _CACHED_NC

    LAST_RESULTS = run_bass_kernel_spmd(nc, in_maps, core_ids=list(range(NCORES)))
    outs = [LAST_RESULTS.results[c]["out"] for c in range(NCORES)]
    return np.concatenate(outs, axis=0).astype(np.float32)


# revision 50
# speedup vs baseline: 1.0899x; 1.0074x over previous
"""MoE actor-critic forward kernel for 8 Trainium2 NeuronCores.

Strategy: data-parallel over the batch axis. Each of the 8 cores gets
B/8 = 2048 tokens plus a full replica of the gating + expert weights.

Per-core math (all activations feature-major [feat_part, tok] in SBUF):
  - gating MLP in fp32 (exact routing): 512 -> 256 -> 128 -> 8 logits
    token-major, then top-2 + renormalized combine weights cw[tok, 8]
    computed with exp/compare/reduce ops (no explicit softmax division
    needed: cw = z * (eq1+eq2) / (1 + m2) with z = exp(l - lmax)).
  - experts in bf16 (fp32 PSUM accumulate): all 8 experts run densely,
    512 -> 1024 -> 512 -> 256 -> 32 with ELU between layers.
    ELU(x) = max(x + b, min(exp(x + b) - 1, 0)) -> 1 ACT + 2 DVE ops.
  - combine: acc[tok, 32] += cw[:, e] * (h3_e @ W4_e + b4_e), where the
    bias enters the matmul via a K=1 ones-row matmul.
"""

import numpy as np
import ml_dtypes

import concourse.bass as bass
import concourse.mybir as mybir
import concourse.tile as tile
from concourse import bacc
from concourse.bass_utils import run_bass_kernel_spmd

BF16 = mybir.dt.bfloat16
F32 = mybir.dt.float32
NP_BF16 = ml_dtypes.bfloat16

B = 16384
D = 512          # obs dim
A = 32           # actions
E = 8            # experts
NCORES = 8
T = B // NCORES  # tokens per core (2048)
NT = T // 512    # 512-token tiles (4)
TT = T // 128    # 128-token tiles (16)

EH1, EH2, EH3 = 1024, 512, 256
GH1, GH2 = 256, 128

LAST_RESULTS = None  # test harness reads exec_time_ns from here


def _build_bass():
    nc = bacc.Bacc("TRN2", target_bir_lowering=False, debug=False,
                   enable_asserts=False, num_devices=NCORES)

    # ---- DRAM I/O ----
    obs_f = nc.dram_tensor("obs_f", [D, T], F32, kind="ExternalInput")
    obs_b = nc.dram_tensor("obs_b", [D, T], BF16, kind="ExternalInput")
    gw1 = nc.dram_tensor("gw1", [D, GH1], F32, kind="ExternalInput")
    gw2 = nc.dram_tensor("gw2", [GH1, GH2], F32, kind="ExternalInput")
    gw3 = nc.dram_tensor("gw3", [GH2, E], F32, kind="ExternalInput")
    gb1 = nc.dram_tensor("gb1", [128, GH1 // 128], F32, kind="ExternalInput")
    gb2 = nc.dram_tensor("gb2", [128, GH2 // 128], F32, kind="ExternalInput")
    gb3 = nc.dram_tensor("gb3", [1, E], F32, kind="ExternalInput")
    ew1 = nc.dram_tensor("ew1", [E, D, EH1], BF16, kind="ExternalInput")
    ew2 = nc.dram_tensor("ew2", [E, EH1, EH2], BF16, kind="ExternalInput")
    ew3 = nc.dram_tensor("ew3", [E, EH2, EH3], BF16, kind="ExternalInput")
    ew4 = nc.dram_tensor("ew4", [E, EH3, A], BF16, kind="ExternalInput")
    eb1 = nc.dram_tensor("eb1", [E, 128, EH1 // 128], F32, kind="ExternalInput")
    eb2 = nc.dram_tensor("eb2", [E, 128, EH2 // 128], F32, kind="ExternalInput")
    eb3 = nc.dram_tensor("eb3", [E, 128, EH3 // 128], F32, kind="ExternalInput")
    eb4 = nc.dram_tensor("eb4", [1, E, A], F32, kind="ExternalInput")
    out = nc.dram_tensor("out", [T, A], F32, kind="ExternalOutput")

    with tile.TileContext(nc) as tc:
        _emit(nc, tc, obs_f, obs_b, gw1, gw2, gw3, gb1, gb2, gb3,
              ew1, ew2, ew3, ew4, eb1, eb2, eb3, eb4, out)
    nc.compile()
    return nc


def _elu(nc, pool, psum, bias_col, h_out):
    """h_out = ELU(psum + bias_col) = max(x+b, min(exp(x+b)-1, 0))."""
    p, n = psum.shape[0], psum.free_size()
    t = pool.tile([128, 512], BF16, tag="elu_t")
    u = pool.tile([128, 512], BF16, tag="elu_u")
    nc.scalar.activation(t[:p, :n], psum, mybir.ActivationFunctionType.Exp,
                         bias=bias_col)
    nc.vector.tensor_scalar(u[:p, :n], t[:p, :n], -1.0, 0.0,
                            mybir.AluOpType.add, mybir.AluOpType.min)
    nc.vector.scalar_tensor_tensor(h_out, psum, bias_col, u[:p, :n],
                                   mybir.AluOpType.add, mybir.AluOpType.max)


def _emit(nc, tc, obs_f, obs_b, gw1, gw2, gw3, gb1, gb2, gb3,
          ew1, ew2, ew3, ew4, eb1, eb2, eb3, eb4, out):
    AF = mybir.ActivationFunctionType
    OP = mybir.AluOpType
    X = mybir.AxisListType.X

    # ---------------- persistent pools ----------------
    from contextlib import ExitStack
    ctx = ExitStack()
    consts = ctx.enter_context(tc.tile_pool(name="consts", bufs=1))
    acts = ctx.enter_context(tc.tile_pool(name="acts", bufs=1))
    wpool = ctx.enter_context(tc.tile_pool(name="wpool", bufs=2))
    tmp = ctx.enter_context(tc.tile_pool(name="tmp", bufs=4))
    psum_mm = ctx.enter_context(tc.tile_pool(name="psum_mm", bufs=6, space="PSUM"))

    # constants
    ones_b = consts.tile([1, 128], BF16)
    nc.vector.memset(ones_b, 1.0)
    ones_f = consts.tile([1, 128], F32)
    nc.vector.memset(ones_f, 1.0)
    b4_sb = consts.tile([1, E, A], F32)
    nc.sync.dma_start(out=b4_sb, in_=eb4[:, :, :])
    gb3_sb = consts.tile([1, E], F32)
    nc.sync.dma_start(out=gb3_sb, in_=gb3[:, :])

    # persistent activations (feature-major, bf16)
    obs_sb = acts.tile([128, D // 128, T], BF16)
    nc.sync.dma_start(out=obs_sb, in_=obs_b.rearrange("(k p) t -> p k t", p=128))
    h1 = acts.tile([128, EH1 // 128, T], BF16)
    h2 = acts.tile([128, EH2 // 128, T], BF16)
    h3 = acts.tile([128, EH3 // 128, T], BF16)
    cw = acts.tile([128, TT, E], F32)          # combine weights, token-major
    # per-token-tile accumulators (separate tiles so the 16 combine chains
    # don't serialize on one tile's write-write deps)
    acc = [acts.tile([128, A], F32, tag=f"acc{t}", name=f"acc{t}")
           for t in range(TT)]

    # ---------------- gating (fp32) ----------------
    with tc.tile_pool(name="gating", bufs=1) as gp, \
         tc.tile_pool(name="gstream", bufs=2) as gs, \
         tc.tile_pool(name="gtmp", bufs=4) as gt, \
         tc.tile_pool(name="psum_g", bufs=1, space="PSUM") as pg:

        gw1_sb = gp.tile([128, D // 128, GH1], F32)
        for k in range(D // 128):
            nc.sync.dma_start(out=gw1_sb[:, k, :],
                              in_=gw1.rearrange("(k p) o -> p k o", p=128)[:, k, :])
        gw2_sb = gp.tile([128, GH1 // 128, GH2], F32)
        nc.sync.dma_start(out=gw2_sb, in_=gw2.rearrange("(k p) o -> p k o", p=128))
        gw3_sb = gp.tile([128, E], F32)
        nc.sync.dma_start(out=gw3_sb, in_=gw3[:, :])
        gb1_sb = gp.tile([128, GH1 // 128], F32)
        nc.sync.dma_start(out=gb1_sb, in_=gb1[:, :])
        gb2_sb = gp.tile([128, GH2 // 128], F32)
        nc.sync.dma_start(out=gb2_sb, in_=gb2[:, :])
        g1 = gp.tile([128, GH1 // 128, T], F32)
        g2 = gp.tile([128, GH2 // 128, T], F32)

        # L1: 512 -> 256 (inputs streamed per K-chunk so the first matmul
        # only waits on one 256KB DMA, not the whole block)
        for n in range(NT):
            obk = []
            for k in range(D // 128):
                ob = gs.tile([128, 512], F32, tag="gobs", bufs=10,
                             name=f"ob{n}_{k}")
                nc.sync.dma_start(
                    out=ob,
                    in_=obs_f.rearrange("(k p) t -> p k t", p=128)[:, k, n * 512:(n + 1) * 512])
                obk.append(ob)
            for m in range(GH1 // 128):
                ps = pg.tile([128, 512], F32, tag="gps")
                for k in range(D // 128):
                    nc.tensor.matmul(ps, gw1_sb[:, k, m * 128:(m + 1) * 128],
                                     obk[k], start=(k == 0), stop=(k == D // 128 - 1))
                _elu_g(nc, gt, ps, gb1_sb[:, m:m + 1], g1[:, m, n * 512:(n + 1) * 512])
        # L2: 256 -> 128
        for n in range(NT):
            ps = pg.tile([128, 512], F32, tag="gps")
            for k in range(GH1 // 128):
                nc.tensor.matmul(ps, gw2_sb[:, k, :], g1[:, k, n * 512:(n + 1) * 512],
                                 start=(k == 0), stop=(k == GH1 // 128 - 1))
            _elu_g(nc, gt, ps, gb2_sb[:, 0:1], g2[:, 0, n * 512:(n + 1) * 512])
        # logits + top-2 combine weights, token-major per 128-token tile
        for t in range(TT):
            pl = pg.tile([128, E], F32, tag="gpl")
            nc.tensor.matmul(pl, g2[:, 0, t * 128:(t + 1) * 128], gw3_sb,
                             start=True, stop=False)
            nc.tensor.matmul(pl, ones_f, gb3_sb, start=False, stop=True)

            mx = gt.tile([128, 1], F32, tag="mx")
            nmx = gt.tile([128, 1], F32, tag="nmx")
            z = gt.tile([128, E], F32, tag="z")
            eq1 = gt.tile([128, E], F32, tag="eq1")
            z2 = gt.tile([128, E], F32, tag="z2")
            m2 = gt.tile([128, 1], F32, tag="m2")
            eq2 = gt.tile([128, E], F32, tag="eq2")
            msk = gt.tile([128, E], F32, tag="msk")
            num = gt.tile([128, E], F32, tag="num")
            den = gt.tile([128, 1], F32, tag="den")
            rec = gt.tile([128, 1], F32, tag="rec")

            nc.vector.reduce_max(out=mx, in_=pl, axis=X)
            nc.vector.tensor_scalar_mul(nmx, mx, -1.0)
            nc.vector.tensor_scalar(eq1, pl, mx, None, OP.is_ge)
            nc.scalar.activation(z, pl, AF.Exp, bias=nmx)
            nc.vector.tensor_sub(z2, z, eq1)
            nc.vector.reduce_max(out=m2, in_=z2, axis=X)
            nc.vector.tensor_scalar(eq2, z2, m2, None, OP.is_ge)
            nc.vector.tensor_add(msk, eq1, eq2)
            nc.vector.tensor_mul(num, z, msk)
            nc.vector.tensor_scalar_add(den, m2, 1.0)
            nc.vector.reciprocal(rec, den)
            nc.vector.tensor_scalar_mul(cw[:, t, :], num, rec)

    # L4 psums get their own pool (reusing the gating pool's PSUM banks) so
    # the next expert's L1 psums don't contend with pending combine reads
    psum_l4 = ctx.enter_context(tc.tile_pool(name="psum_l4", bufs=2, space="PSUM"))

    # ---------------- experts (bf16) ----------------
    for e in range(E):
        w1_sb = wpool.tile([128, D // 128, EH1], BF16, tag="w1")
        nc.sync.dma_start(out=w1_sb, in_=ew1[e].rearrange("(k p) o -> p k o", p=128))
        w2_sb = wpool.tile([128, EH1 // 128, EH2], BF16, tag="w2")
        nc.sync.dma_start(out=w2_sb, in_=ew2[e].rearrange("(k p) o -> p k o", p=128))
        w3_sb = wpool.tile([128, EH2 // 128, EH3], BF16, tag="w3")
        nc.sync.dma_start(out=w3_sb, in_=ew3[e].rearrange("(k p) o -> p k o", p=128))
        w4_sb = wpool.tile([128, EH3 // 128, A], BF16, tag="w4")
        nc.sync.dma_start(out=w4_sb, in_=ew4[e].rearrange("(k p) o -> p k o", p=128))
        b1_sb = wpool.tile([128, EH1 // 128], F32, tag="b1")
        nc.sync.dma_start(out=b1_sb, in_=eb1[e])
        b2_sb = wpool.tile([128, EH2 // 128], F32, tag="b2")
        nc.sync.dma_start(out=b2_sb, in_=eb2[e])
        b3_sb = wpool.tile([128, EH3 // 128], F32, tag="b3")
        nc.sync.dma_start(out=b3_sb, in_=eb3[e])

        # L1: 512 -> 1024
        for n in range(NT):
            for m in range(EH1 // 128):
                ps = psum_mm.tile([128, 512], F32, tag="mm")
                for k in range(D // 128):
                    nc.tensor.matmul(ps, w1_sb[:, k, m * 128:(m + 1) * 128],
                                     obs_sb[:, k, n * 512:(n + 1) * 512],
                                     start=(k == 0), stop=(k == D // 128 - 1))
                _elu(nc, tmp, ps, b1_sb[:, m:m + 1], h1[:, m, n * 512:(n + 1) * 512])
        # L2: 1024 -> 512
        for n in range(NT):
            for m in range(EH2 // 128):
                ps = psum_mm.tile([128, 512], F32, tag="mm")
                for k in range(EH1 // 128):
                    nc.tensor.matmul(ps, w2_sb[:, k, m * 128:(m + 1) * 128],
                                     h1[:, k, n * 512:(n + 1) * 512],
                                     start=(k == 0), stop=(k == EH1 // 128 - 1))
                _elu(nc, tmp, ps, b2_sb[:, m:m + 1], h2[:, m, n * 512:(n + 1) * 512])
        # L3: 512 -> 256, with L4 + weighted combine (256 -> 32, token-major)
        # interleaved per n-block so the combine overlaps later L3 blocks
        # instead of being exposed at the end of the last expert
        for n in range(NT):
            for m in range(EH3 // 128):
                ps = psum_mm.tile([128, 512], F32, tag="mm")
                for k in range(EH2 // 128):
                    nc.tensor.matmul(ps, w3_sb[:, k, m * 128:(m + 1) * 128],
                                     h2[:, k, n * 512:(n + 1) * 512],
                                     start=(k == 0), stop=(k == EH2 // 128 - 1))
                _elu(nc, tmp, ps, b3_sb[:, m:m + 1], h3[:, m, n * 512:(n + 1) * 512])
            for t in range(4 * n, 4 * n + 4):
                p4 = psum_l4.tile([128, A], F32, tag="l4")
                for k in range(EH3 // 128):
                    nc.tensor.matmul(p4, h3[:, k, t * 128:(t + 1) * 128],
                                     w4_sb[:, k, :], start=(k == 0), stop=False)
                nc.tensor.matmul(p4, ones_f, b4_sb[:, e, :], start=False, stop=True)
                if e == 0:
                    nc.vector.tensor_scalar(acc[t], p4, cw[:, t, e:e + 1], None,
                                            OP.mult)
                else:
                    nc.vector.scalar_tensor_tensor(acc[t], p4, cw[:, t, e:e + 1],
                                                   acc[t], OP.mult, OP.add)

    # ---------------- store ----------------
    for t in range(TT):
        nc.sync.dma_start(
            out=out.rearrange("(t p) a -> p t a", p=128)[:, t, :], in_=acc[t])

    ctx.close()


def _elu_g(nc, pool, psum, bias_col, h_out):
    """fp32 ELU for the gating net."""
    t = pool.tile([128, 512], F32, tag="gelu_t")
    u = pool.tile([128, 512], F32, tag="gelu_u")
    n = psum.free_size()
    nc.scalar.activation(t[:, :n], psum, mybir.ActivationFunctionType.Exp,
                         bias=bias_col)
    nc.vector.tensor_scalar(u[:, :n], t[:, :n], -1.0, 0.0,
                            mybir.AluOpType.add, mybir.AluOpType.min)
    nc.vector.scalar_tensor_tensor(h_out, psum, bias_col, u[:, :n],
                                   mybir.AluOpType.add, mybir.AluOpType.max)


_CACHED_NC = None


def kernel(**inputs) -> np.ndarray:
    global LAST_RESULTS, _CACHED_NC
    obs = np.ascontiguousarray(inputs["observations"], dtype=np.float32)

    def pp_bias(b):  # [chunks*128] -> [128, chunks] per-partition layout
        c = b.shape[-1] // 128
        return np.ascontiguousarray(
            b.reshape(b.shape[:-1] + (c, 128)).swapaxes(-1, -2), dtype=np.float32)

    gw1 = np.asarray(inputs["gw1"], np.float32)
    gw2 = np.asarray(inputs["gw2"], np.float32)
    gw3 = np.asarray(inputs["gw3"], np.float32)
    gb1 = pp_bias(np.asarray(inputs["gb1"], np.float32))
    gb2 = pp_bias(np.asarray(inputs["gb2"], np.float32))
    gb3 = np.asarray(inputs["gb3"], np.float32).reshape(1, E)
    ew1 = np.ascontiguousarray(inputs["ew1"], dtype=np.float32).astype(NP_BF16)
    ew2 = np.ascontiguousarray(inputs["ew2"], dtype=np.float32).astype(NP_BF16)
    ew3 = np.ascontiguousarray(inputs["ew3"], dtype=np.float32).astype(NP_BF16)
    ew4 = np.ascontiguousarray(inputs["ew4"], dtype=np.float32).astype(NP_BF16)
    eb1 = pp_bias(np.asarray(inputs["eb1"], np.float32))
    eb2 = pp_bias(np.asarray(inputs["eb2"], np.float32))
    eb3 = pp_bias(np.asarray(inputs["eb3"], np.float32))
    eb4 = np.ascontiguousarray(np.asarray(inputs["eb4"], np.float32).reshape(1, E, A))

    shared = {
        "gw1": gw1, "gw2": gw2, "gw3": gw3,
        "gb1": gb1, "gb2": gb2, "gb3": gb3,
        "ew1": ew1, "ew2": ew2, "ew3": ew3, "ew4": ew4,
        "eb1": eb1, "eb2": eb2, "eb3": eb3, "eb4": eb4,
    }
    in_maps = []
    for c in range(NCORES):
        sl = obs[c * T:(c + 1) * T]                    # [T, D]
        obs_t = np.ascontiguousarray(sl.T)             # [D, T] fp32
        m = dict(shared)
        m["obs_f"] = obs_t
        m["obs_b"] = obs_t.astype(NP_BF16)
        in_maps.append(m)

    if _CACHED_NC is None:
        _CACHED_NC = _build_bass()
    nc = _CACHED_NC

    LAST_RESULTS = run_bass_kernel_spmd(nc, in_maps, core_ids=list(range(NCORES)))
    outs = [LAST_RESULTS.results[c]["out"] for c in range(NCORES)]
    return np.concatenate(outs, axis=0).astype(np.float32)


# revision 59
# speedup vs baseline: 1.0969x; 1.0065x over previous
"""MoE actor-critic forward kernel for 8 Trainium2 NeuronCores.

Strategy: data-parallel over the batch axis. Each of the 8 cores gets
B/8 = 2048 tokens plus a full replica of the gating + expert weights.

Per-core math (all activations feature-major [feat_part, tok] in SBUF):
  - gating MLP in fp32 (exact routing): 512 -> 256 -> 128 -> 8 logits
    token-major, then top-2 + renormalized combine weights cw[tok, 8]
    computed with exp/compare/reduce ops (no explicit softmax division
    needed: cw = z * (eq1+eq2) / (1 + m2) with z = exp(l - lmax)).
  - experts in bf16 (fp32 PSUM accumulate): all 8 experts run densely,
    512 -> 1024 -> 512 -> 256 -> 32 with ELU between layers.
    ELU(x) = max(x + b, min(exp(x + b) - 1, 0)) -> 1 ACT + 2 DVE ops.
  - combine: acc[tok, 32] += cw[:, e] * (h3_e @ W4_e + b4_e), where the
    bias enters the matmul via a K=1 ones-row matmul.
"""

import numpy as np
import ml_dtypes

import concourse.bass as bass
import concourse.mybir as mybir
import concourse.tile as tile
from concourse import bacc
from concourse.bass_utils import run_bass_kernel_spmd

BF16 = mybir.dt.bfloat16
F32 = mybir.dt.float32
NP_BF16 = ml_dtypes.bfloat16

B = 16384
D = 512          # obs dim
A = 32           # actions
E = 8            # experts
NCORES = 8
T = B // NCORES  # tokens per core (2048)
NT = T // 512    # 512-token tiles (4)
TT = T // 128    # 128-token tiles (16)

EH1, EH2, EH3 = 1024, 512, 256
GH1, GH2 = 256, 128

LAST_RESULTS = None  # test harness reads exec_time_ns from here


def _build_bass():
    nc = bacc.Bacc("TRN2", target_bir_lowering=False, debug=False,
                   enable_asserts=False, num_devices=NCORES)

    # ---- DRAM I/O ----
    obs_f = nc.dram_tensor("obs_f", [D, T], F32, kind="ExternalInput")
    obs_b = nc.dram_tensor("obs_b", [D, T], BF16, kind="ExternalInput")
    gw1 = nc.dram_tensor("gw1", [D, GH1], F32, kind="ExternalInput")
    gw2 = nc.dram_tensor("gw2", [GH1, GH2], F32, kind="ExternalInput")
    gw3 = nc.dram_tensor("gw3", [GH2, E], F32, kind="ExternalInput")
    gb1 = nc.dram_tensor("gb1", [128, GH1 // 128], F32, kind="ExternalInput")
    gb2 = nc.dram_tensor("gb2", [128, GH2 // 128], F32, kind="ExternalInput")
    gb3 = nc.dram_tensor("gb3", [1, E], F32, kind="ExternalInput")
    ew1 = nc.dram_tensor("ew1", [E, D, EH1], BF16, kind="ExternalInput")
    ew2 = nc.dram_tensor("ew2", [E, EH1, EH2], BF16, kind="ExternalInput")
    ew3 = nc.dram_tensor("ew3", [E, EH2, EH3], BF16, kind="ExternalInput")
    ew4 = nc.dram_tensor("ew4", [E, EH3, A], BF16, kind="ExternalInput")
    eb1 = nc.dram_tensor("eb1", [E, 128, EH1 // 128], F32, kind="ExternalInput")
    eb2 = nc.dram_tensor("eb2", [E, 128, EH2 // 128], F32, kind="ExternalInput")
    eb3 = nc.dram_tensor("eb3", [E, 128, EH3 // 128], F32, kind="ExternalInput")
    eb4 = nc.dram_tensor("eb4", [1, E, A], BF16, kind="ExternalInput")
    out = nc.dram_tensor("out", [T, A], F32, kind="ExternalOutput")

    with tile.TileContext(nc) as tc:
        _emit(nc, tc, obs_f, obs_b, gw1, gw2, gw3, gb1, gb2, gb3,
              ew1, ew2, ew3, ew4, eb1, eb2, eb3, eb4, out)
    nc.compile()
    return nc


def _elu(nc, pool, psum, bias_col, h_out):
    """h_out = ELU(psum + bias_col) = max(x+b, min(exp(x+b)-1, 0))."""
    p, n = psum.shape[0], psum.free_size()
    t = pool.tile([128, 512], BF16, tag="elu_t")
    u = pool.tile([128, 512], BF16, tag="elu_u")
    nc.scalar.activation(t[:p, :n], psum, mybir.ActivationFunctionType.Exp,
                         bias=bias_col)
    nc.vector.tensor_scalar(u[:p, :n], t[:p, :n], -1.0, 0.0,
                            mybir.AluOpType.add, mybir.AluOpType.min)
    nc.vector.scalar_tensor_tensor(h_out, psum, bias_col, u[:p, :n],
                                   mybir.AluOpType.add, mybir.AluOpType.max)


def _emit(nc, tc, obs_f, obs_b, gw1, gw2, gw3, gb1, gb2, gb3,
          ew1, ew2, ew3, ew4, eb1, eb2, eb3, eb4, out):
    AF = mybir.ActivationFunctionType
    OP = mybir.AluOpType
    X = mybir.AxisListType.X

    # ---------------- persistent pools ----------------
    from contextlib import ExitStack
    ctx = ExitStack()
    consts = ctx.enter_context(tc.tile_pool(name="consts", bufs=1))
    acts = ctx.enter_context(tc.tile_pool(name="acts", bufs=1))
    wpool = ctx.enter_context(tc.tile_pool(name="wpool", bufs=2))
    tmp = ctx.enter_context(tc.tile_pool(name="tmp", bufs=4))
    psum_mm = ctx.enter_context(tc.tile_pool(name="psum_mm", bufs=6, space="PSUM"))

    # constants
    ones_b = consts.tile([1, 128], BF16)
    nc.vector.memset(ones_b, 1.0)
    ones_f = consts.tile([1, 128], F32)
    nc.vector.memset(ones_f, 1.0)
    b4_sb = consts.tile([1, E, A], BF16)
    nc.sync.dma_start(out=b4_sb, in_=eb4[:, :, :])
    gb3_sb = consts.tile([1, E], F32)
    nc.sync.dma_start(out=gb3_sb, in_=gb3[:, :])

    # persistent activations (feature-major, bf16)
    obs_sb = acts.tile([128, D // 128, T], BF16)
    nc.sync.dma_start(out=obs_sb, in_=obs_b.rearrange("(k p) t -> p k t", p=128))
    h1 = acts.tile([128, EH1 // 128, T], BF16)
    h2 = acts.tile([128, EH2 // 128, T], BF16)
    h3 = acts.tile([128, EH3 // 128, T], BF16)
    cw = acts.tile([128, TT, E], F32)          # combine weights, token-major
    # per-token-tile accumulators (separate tiles so the 16 combine chains
    # don't serialize on one tile's write-write deps)
    acc = [acts.tile([128, A], F32, tag=f"acc{t}", name=f"acc{t}")
           for t in range(TT)]

    # ---------------- gating (fp32) ----------------
    with tc.tile_pool(name="gating", bufs=1) as gp, \
         tc.tile_pool(name="gstream", bufs=2) as gs, \
         tc.tile_pool(name="gtmp", bufs=4) as gt, \
         tc.tile_pool(name="psum_g", bufs=1, space="PSUM") as pg:

        gw1_sb = gp.tile([128, D // 128, GH1], F32)
        for k in range(D // 128):
            nc.sync.dma_start(out=gw1_sb[:, k, :],
                              in_=gw1.rearrange("(k p) o -> p k o", p=128)[:, k, :])
        gw2_sb = gp.tile([128, GH1 // 128, GH2], F32)
        nc.sync.dma_start(out=gw2_sb, in_=gw2.rearrange("(k p) o -> p k o", p=128))
        gw3_sb = gp.tile([128, E], F32)
        nc.sync.dma_start(out=gw3_sb, in_=gw3[:, :])
        gb1_sb = gp.tile([128, GH1 // 128], F32)
        nc.sync.dma_start(out=gb1_sb, in_=gb1[:, :])
        gb2_sb = gp.tile([128, GH2 // 128], F32)
        nc.sync.dma_start(out=gb2_sb, in_=gb2[:, :])
        g1 = gp.tile([128, GH1 // 128, T], F32)
        g2 = gp.tile([128, GH2 // 128, T], F32)

        # L1: 512 -> 256 (inputs streamed per K-chunk so the first matmul
        # only waits on one 256KB DMA, not the whole block)
        for n in range(NT):
            obk = []
            for k in range(D // 128):
                ob = gs.tile([128, 512], F32, tag="gobs", bufs=10,
                             name=f"ob{n}_{k}")
                nc.sync.dma_start(
                    out=ob,
                    in_=obs_f.rearrange("(k p) t -> p k t", p=128)[:, k, n * 512:(n + 1) * 512])
                obk.append(ob)
            for m in range(GH1 // 128):
                ps = pg.tile([128, 512], F32, tag="gps")
                for k in range(D // 128):
                    nc.tensor.matmul(ps, gw1_sb[:, k, m * 128:(m + 1) * 128],
                                     obk[k], start=(k == 0), stop=(k == D // 128 - 1))
                _elu_g(nc, gt, ps, gb1_sb[:, m:m + 1], g1[:, m, n * 512:(n + 1) * 512])
        # L2: 256 -> 128
        for n in range(NT):
            ps = pg.tile([128, 512], F32, tag="gps")
            for k in range(GH1 // 128):
                nc.tensor.matmul(ps, gw2_sb[:, k, :], g1[:, k, n * 512:(n + 1) * 512],
                                 start=(k == 0), stop=(k == GH1 // 128 - 1))
            _elu_g(nc, gt, ps, gb2_sb[:, 0:1], g2[:, 0, n * 512:(n + 1) * 512])
        # logits + top-2 combine weights, token-major per 128-token tile
        for t in range(TT):
            pl = pg.tile([128, E], F32, tag="gpl")
            nc.tensor.matmul(pl, g2[:, 0, t * 128:(t + 1) * 128], gw3_sb,
                             start=True, stop=False)
            nc.tensor.matmul(pl, ones_f, gb3_sb, start=False, stop=True)

            mx = gt.tile([128, 1], F32, tag="mx")
            nmx = gt.tile([128, 1], F32, tag="nmx")
            z = gt.tile([128, E], F32, tag="z")
            eq1 = gt.tile([128, E], F32, tag="eq1")
            z2 = gt.tile([128, E], F32, tag="z2")
            m2 = gt.tile([128, 1], F32, tag="m2")
            eq2 = gt.tile([128, E], F32, tag="eq2")
            msk = gt.tile([128, E], F32, tag="msk")
            num = gt.tile([128, E], F32, tag="num")
            den = gt.tile([128, 1], F32, tag="den")
            rec = gt.tile([128, 1], F32, tag="rec")

            nc.vector.reduce_max(out=mx, in_=pl, axis=X)
            nc.vector.tensor_scalar_mul(nmx, mx, -1.0)
            nc.vector.tensor_scalar(eq1, pl, mx, None, OP.is_ge)
            nc.scalar.activation(z, pl, AF.Exp, bias=nmx)
            nc.vector.tensor_sub(z2, z, eq1)
            nc.vector.reduce_max(out=m2, in_=z2, axis=X)
            nc.vector.tensor_scalar(eq2, z2, m2, None, OP.is_ge)
            nc.vector.tensor_add(msk, eq1, eq2)
            nc.vector.tensor_mul(num, z, msk)
            nc.vector.tensor_scalar_add(den, m2, 1.0)
            nc.vector.reciprocal(rec, den)
            nc.vector.tensor_scalar_mul(cw[:, t, :], num, rec)

    # L4 psums get their own pool (reusing the gating pool's PSUM banks) so
    # the next expert's L1 psums don't contend with pending combine reads
    psum_l4 = ctx.enter_context(tc.tile_pool(name="psum_l4", bufs=2, space="PSUM"))

    # ---------------- experts (bf16) ----------------
    for e in range(E):
        w1_sb = wpool.tile([128, D // 128, EH1], BF16, tag="w1")
        nc.sync.dma_start(out=w1_sb, in_=ew1[e].rearrange("(k p) o -> p k o", p=128))
        w2_sb = wpool.tile([128, EH1 // 128, EH2], BF16, tag="w2")
        nc.sync.dma_start(out=w2_sb, in_=ew2[e].rearrange("(k p) o -> p k o", p=128))
        w3_sb = wpool.tile([128, EH2 // 128, EH3], BF16, tag="w3")
        nc.sync.dma_start(out=w3_sb, in_=ew3[e].rearrange("(k p) o -> p k o", p=128))
        w4_sb = wpool.tile([128, EH3 // 128, A], BF16, tag="w4")
        nc.sync.dma_start(out=w4_sb, in_=ew4[e].rearrange("(k p) o -> p k o", p=128))
        b1_sb = wpool.tile([128, EH1 // 128], F32, tag="b1")
        nc.sync.dma_start(out=b1_sb, in_=eb1[e])
        b2_sb = wpool.tile([128, EH2 // 128], F32, tag="b2")
        nc.sync.dma_start(out=b2_sb, in_=eb2[e])
        b3_sb = wpool.tile([128, EH3 // 128], F32, tag="b3")
        nc.sync.dma_start(out=b3_sb, in_=eb3[e])

        # L1: 512 -> 1024
        for n in range(NT):
            for m in range(EH1 // 128):
                ps = psum_mm.tile([128, 512], F32, tag="mm")
                for k in range(D // 128):
                    nc.tensor.matmul(ps, w1_sb[:, k, m * 128:(m + 1) * 128],
                                     obs_sb[:, k, n * 512:(n + 1) * 512],
                                     start=(k == 0), stop=(k == D // 128 - 1))
                _elu(nc, tmp, ps, b1_sb[:, m:m + 1], h1[:, m, n * 512:(n + 1) * 512])
        # L2: 1024 -> 512
        for n in range(NT):
            for m in range(EH2 // 128):
                ps = psum_mm.tile([128, 512], F32, tag="mm")
                for k in range(EH1 // 128):
                    nc.tensor.matmul(ps, w2_sb[:, k, m * 128:(m + 1) * 128],
                                     h1[:, k, n * 512:(n + 1) * 512],
                                     start=(k == 0), stop=(k == EH1 // 128 - 1))
                _elu(nc, tmp, ps, b2_sb[:, m:m + 1], h2[:, m, n * 512:(n + 1) * 512])
        # L3: 512 -> 256, with L4 + weighted combine (256 -> 32, token-major)
        # interleaved per n-block so the combine overlaps later L3 blocks
        # instead of being exposed at the end of the last expert
        for n in range(NT):
            for m in range(EH3 // 128):
                ps = psum_mm.tile([128, 512], F32, tag="mm")
                for k in range(EH2 // 128):
                    nc.tensor.matmul(ps, w3_sb[:, k, m * 128:(m + 1) * 128],
                                     h2[:, k, n * 512:(n + 1) * 512],
                                     start=(k == 0), stop=(k == EH2 // 128 - 1))
                _elu(nc, tmp, ps, b3_sb[:, m:m + 1], h3[:, m, n * 512:(n + 1) * 512])
            for t in range(4 * n, 4 * n + 4):
                p4 = psum_l4.tile([128, A], F32, tag="l4")
                for k in range(EH3 // 128):
                    nc.tensor.matmul(p4, h3[:, k, t * 128:(t + 1) * 128],
                                     w4_sb[:, k, :], start=(k == 0), stop=False)
                nc.tensor.matmul(p4, ones_b, b4_sb[:, e, :], start=False, stop=True)
                if e == 0:
                    nc.vector.tensor_scalar(acc[t], p4, cw[:, t, e:e + 1], None,
                                            OP.mult)
                else:
                    nc.vector.scalar_tensor_tensor(acc[t], p4, cw[:, t, e:e + 1],
                                                   acc[t], OP.mult, OP.add)

    # ---------------- store ----------------
    for t in range(TT):
        nc.sync.dma_start(
            out=out.rearrange("(t p) a -> p t a", p=128)[:, t, :], in_=acc[t])

    ctx.close()


def _elu_g(nc, pool, psum, bias_col, h_out):
    """fp32 ELU for the gating net."""
    t = pool.tile([128, 512], F32, tag="gelu_t")
    u = pool.tile([128, 512], F32, tag="gelu_u")
    n = psum.free_size()
    nc.scalar.activation(t[:, :n], psum, mybir.ActivationFunctionType.Exp,
                         bias=bias_col)
    nc.vector.tensor_scalar(u[:, :n], t[:, :n], -1.0, 0.0,
                            mybir.AluOpType.add, mybir.AluOpType.min)
    nc.vector.scalar_tensor_tensor(h_out, psum, bias_col, u[:, :n],
                                   mybir.AluOpType.add, mybir.AluOpType.max)


_CACHED_NC = None


def kernel(**inputs) -> np.ndarray:
    global LAST_RESULTS, _CACHED_NC
    obs = np.ascontiguousarray(inputs["observations"], dtype=np.float32)

    def pp_bias(b):  # [chunks*128] -> [128, chunks] per-partition layout
        c = b.shape[-1] // 128
        return np.ascontiguousarray(
            b.reshape(b.shape[:-1] + (c, 128)).swapaxes(-1, -2), dtype=np.float32)

    gw1 = np.asarray(inputs["gw1"], np.float32)
    gw2 = np.asarray(inputs["gw2"], np.float32)
    gw3 = np.asarray(inputs["gw3"], np.float32)
    gb1 = pp_bias(np.asarray(inputs["gb1"], np.float32))
    gb2 = pp_bias(np.asarray(inputs["gb2"], np.float32))
    gb3 = np.asarray(inputs["gb3"], np.float32).reshape(1, E)
    ew1 = np.ascontiguousarray(inputs["ew1"], dtype=np.float32).astype(NP_BF16)
    ew2 = np.ascontiguousarray(inputs["ew2"], dtype=np.float32).astype(NP_BF16)
    ew3 = np.ascontiguousarray(inputs["ew3"], dtype=np.float32).astype(NP_BF16)
    ew4 = np.ascontiguousarray(inputs["ew4"], dtype=np.float32).astype(NP_BF16)
    eb1 = pp_bias(np.asarray(inputs["eb1"], np.float32))
    eb2 = pp_bias(np.asarray(inputs["eb2"], np.float32))
    eb3 = pp_bias(np.asarray(inputs["eb3"], np.float32))
    eb4 = np.asarray(inputs["eb4"], np.float32).reshape(1, E, A).astype(NP_BF16)

    shared = {
        "gw1": gw1, "gw2": gw2, "gw3": gw3,
        "gb1": gb1, "gb2": gb2, "gb3": gb3,
        "ew1": ew1, "ew2": ew2, "ew3": ew3, "ew4": ew4,
        "eb1": eb1, "eb2": eb2, "eb3": eb3, "eb4": eb4,
    }
    in_maps = []
    for c in range(NCORES):
        sl = obs[c * T:(c + 1) * T]                    # [T, D]
        obs_t = np.ascontiguousarray(sl.T)             # [D, T] fp32
        m = dict(shared)
        m["obs_f"] = obs_t
        m["obs_b"] = obs_t.astype(NP_BF16)
        in_maps.append(m)

    if _CACHED_NC is None:
        _CACHED_NC = _build_bass()
    nc = _CACHED_NC

    LAST_RESULTS = run_bass_kernel_spmd(nc, in_maps, core_ids=list(range(NCORES)))
    outs = [LAST_RESULTS.results[c]["out"] for c in range(NCORES)]
    return np.concatenate(outs, axis=0).astype(np.float32)
